# revision 11
# baseline (speedup 1.0000x reference)
"""Trainium2 Bass kernel for a 2-layer BCos-GCN (nn_BCosGCN_28346784153649).

Strategy (8 NeuronCores, SPMD):
  - Nodes (and their incident edges, grouped by destination block) are
    sharded across the 8 cores; the 128x128 weights are replicated.
  - Layer 1: the projection (dinv*x) @ W1 is REPLICATED on every core from a
    host-staged fp16 x^T (rotated per core so each core's own nodes occupy
    table rows [0, 3200) per residue bank) -- no collective needed.
  - Aggregation per layer: each core gathers source rows with dma_gather
    (int16 indices into 4 "residue bank" views of the fp16 table) and
    accumulates per 128-destination-node block via one-hot matmuls (PSUM
    accumulation).  The one-hot S matrices are built ON-CHIP with a single
    is_equal per block against an iota constant (compact per-edge slot ids
    are the only HBM traffic).
  - LayerNorm scale-invariance: since b1/b2 are zero, the dinv[dst] scaling
    before LN is folded away exactly by using a per-node eps*deg bias inside
    the variance sqrt.
  - Layer 2 projection runs inside the layer-1 loop per 4-block group; a
    4-way AllGather (one per residue bank, Shared outputs) assembles the
    rank-major layer-2 table; the layer-2 self rows stay resident in SBUF.
  - Global mean-pool via one-hot matmul accumulation + weight-normalized
    classifier; tiny [128, 10] per-core partials are combined on the host.
"""

import sys

sys.path.insert(0, "/opt/trn_rl_repo")

import numpy as np

from concourse import bacc, tile, mybir
from concourse.bass_utils import run_bass_kernel_spmd
from concourse.masks import make_identity

# ---------------------------------------------------------------- constants
N, E, F, H, C, G = 100000, 1600000, 128, 128, 10, 512
LN_EPS = 1e-5
BCOS_EPS = 1e-6
TEMP = 1.5
RR = 0.6  # residual ratio; bcos exponent B == 1.0 -> bcos(h) = TEMP*h/(nrm+eps)

NCORES = 8
P = 128
REAL_PER_CORE = N // NCORES            # 12500
NODES_PER_CORE = 12800                 # padded: 100 blocks of 128
BLOCKS_PER_CORE = NODES_PER_CORE // P  # 100
NPAD = NODES_PER_CORE * NCORES         # 102400
NBLK = NPAD // P                       # 800
RES = 4                                # residue banks (slot // 32)
B_GRP = 4                              # dst blocks per gather call / group
N_GRP = BLOCKS_PER_CORE // B_GRP       # 25 groups per core
CL_PC = NODES_PER_CORE // RES          # 3200 residue-table rows per core
PJ = 4                                 # blocks per proj1 iteration

F16 = mybir.dt.float16
F32 = mybir.dt.float32
I16 = mybir.dt.int16
I32 = mybir.dt.int32
AOp = mybir.AluOpType
Act = mybir.ActivationFunctionType
AxX = mybir.AxisListType.X


# ---------------------------------------------------------------- host prep
def _lpt_blocks(indeg_core: np.ndarray) -> list[list[int]]:
    """Pack the core's real nodes into 100 blocks of <=128, balancing the
    in-degree sum per block (greedy LPT)."""
    import heapq

    order = np.argsort(-indeg_core, kind="stable")
    heap = [(0, 0, b) for b in range(BLOCKS_PER_CORE)]
    heapq.heapify(heap)
    blocks: list[list[int]] = [[] for _ in range(BLOCKS_PER_CORE)]
    for v in order:
        while True:
            load, cnt, b = heapq.heappop(heap)
            if cnt < P:
                break
        blocks[b].append(int(v))
        heapq.heappush(heap, (load + int(indeg_core[v]), cnt + 1, b))
    return blocks


def _color_banks(ownblk, src, dstblk, rounds=24, seed=0):
    """Greedy residue-bank coloring balancing (dst-block, color) edge cells
    at <=512 (-> K=4), subject to <=32 nodes per (own-block, color)."""
    SLOT_CAP = P // RES
    Nn = ownblk.shape[0]
    rng = np.random.default_rng(seed)
    eorder = np.argsort(src, kind="stable")
    e_dstblk = dstblk[eorder]
    esrc = src[eorder]
    degn = np.bincount(src, minlength=Nn)
    estart = np.concatenate([[0], np.cumsum(degn)])
    cellcnt = np.zeros((NBLK, RES), np.int64)
    slotcnt = np.zeros((NBLK, RES), np.int32)
    color = np.full(Nn, -1, np.int32)
    order = np.argsort(-degn, kind="stable")
    target = max(1.0, dstblk.shape[0] / (NBLK * RES))
    cap = int(np.ceil(target / P) * P)
    for bt in np.array_split(order, rounds):
        nb = bt.shape[0]
        reps = degn[bt]
        node_rep = np.repeat(np.arange(nb), reps)
        eidx = (np.concatenate([np.arange(estart[v], estart[v + 1]) for v in bt])
                if nb else np.empty(0, np.int64))
        score = np.zeros((nb, RES), np.float64)
        if eidx.size:
            np.add.at(score, node_rep, cellcnt[e_dstblk[eidx]])
        own = ownblk[bt]
        score += np.where(slotcnt[own] >= SLOT_CAP, 1e12, 0.0)
        if eidx.size:
            np.add.at(score, node_rep,
                      np.where(cellcnt[e_dstblk[eidx]] >= cap - 1, 1e6, 0.0))
        score += rng.random((nb, RES))
        ch = np.argmin(score, axis=1).astype(np.int32)
        for i in range(nb):
            o, c = own[i], ch[i]
            if slotcnt[o, c] >= SLOT_CAP:
                c = int(np.argmin(slotcnt[o] + np.where(
                    slotcnt[o] >= SLOT_CAP, 10**9, 0)))
                ch[i] = c
            slotcnt[o, c] += 1
        color[bt] = ch
        if eidx.size:
            np.add.at(cellcnt, (e_dstblk[eidx], ch[node_rep]), 1)
    # exact repair: move nodes out of over-cap cells
    border = np.argsort(e_dstblk, kind="stable")
    bcnt = np.bincount(e_dstblk, minlength=NBLK)
    bstart = np.concatenate([[0], np.cumsum(bcnt)])
    for _ in range(40):
        over = np.argwhere(cellcnt > cap)
        if over.size == 0:
            break
        for bb, cc in over:
            while cellcnt[bb, cc] > cap:
                cands = np.unique(esrc[border[bstart[bb]:bstart[bb + 1]]])
                cands = cands[color[cands] == cc]
                moved = False
                contrib = np.array([
                    np.count_nonzero(e_dstblk[estart[v]:estart[v + 1]] == bb)
                    for v in cands])
                for v in cands[np.argsort(contrib)]:
                    o = ownblk[v]
                    blks = e_dstblk[estart[v]:estart[v + 1]]
                    for c2 in np.argsort(cellcnt[bb]):
                        if c2 == cc or slotcnt[o, c2] >= SLOT_CAP:
                            continue
                        add = np.bincount(blks, minlength=NBLK)
                        touched = np.nonzero(add)[0]
                        if (cellcnt[touched, c2] + add[touched] <= cap).all():
                            cellcnt[touched, cc] -= add[touched]
                            cellcnt[touched, c2] += add[touched]
                            slotcnt[o, cc] -= 1
                            slotcnt[o, c2] += 1
                            color[v] = c2
                            moved = True
                            break
                    if moved:
                        break
                if not moved:
                    break
    return color


def _prep(x, src, dst, batch, W1, b1, ln1_w, ln1_b, W2, b2, ln2_w, ln2_b,
          cls_v, cls_g, cls_b, seed=0):
    rng = np.random.default_rng(seed)

    indeg = np.bincount(dst, minlength=N)
    deg = indeg.astype(np.float32) + 1.0
    dinv = (1.0 / np.sqrt(deg)).astype(np.float32)

    # ---- node -> (core, block); LPT balance in-degree per block
    ownblk = np.zeros(N, np.int64)
    core_blocks = []
    g_base = np.zeros(NCORES, np.int64)
    for c in range(NCORES):
        lo, hi = c * REAL_PER_CORE, (c + 1) * REAL_PER_CORE
        g_base[c] = int(batch[lo])
        span = int(batch[hi - 1]) - g_base[c]
        assert span < P, f"core {c} spans {span + 1} graphs > 128"
        blocks = _lpt_blocks(indeg[lo:hi])
        core_blocks.append(blocks)
        for b in range(BLOCKS_PER_CORE):
            for v_local in blocks[b]:
                ownblk[lo + v_local] = c * BLOCKS_PER_CORE + b

    # ---- residue-bank coloring (cells <= 512 -> K=4); slot assignment
    s64 = src.astype(np.int64)
    d64 = dst.astype(np.int64)
    color = _color_banks(ownblk, s64, ownblk[d64])
    pos = np.full(N, -1, np.int64)
    for c in range(NCORES):
        lo = c * REAL_PER_CORE
        for b in range(BLOCKS_PER_CORE):
            blk = core_blocks[c][b]
            base = c * NODES_PER_CORE + b * P
            # color r occupies contiguous slots [32r, 32r+31] so each
            # residue class is a contiguous partition range
            nxt = [0, 0, 0, 0]
            for v_local in blk:
                cc = int(color[lo + v_local])
                sl = 32 * cc + nxt[cc]
                nxt[cc] += 1
                pos[lo + v_local] = base + sl
    assert (pos >= 0).all()

    # ---- per-position node data (pad positions keep zeros / neutral values)
    node_at = np.full(NPAD, -1, np.int64)
    node_at[pos] = np.arange(N)
    ok = node_at >= 0
    sel = node_at[ok]

    # x^T, prescaled by dinv; per-core rotated copies (own nodes first)
    xpos = np.zeros((NPAD, F), np.float32)
    xpos[ok] = x[sel] * dinv[sel][:, None]
    xsTg = np.ascontiguousarray(xpos.T.astype(np.float16))       # [F, NPAD]
    xsT = [np.ascontiguousarray(np.roll(xsTg, -NODES_PER_CORE * c, axis=1))
           for c in range(NCORES)]

    degpos = np.ones(NPAD, np.float32)
    degpos[ok] = deg[sel]
    batpos = np.zeros(NPAD, np.float32)
    batpos[ok] = batch[sel].astype(np.float32)

    trivial = dict(
        b1=not np.any(b1), b2=not np.any(b2),
        ln1=bool(np.all(ln1_w == 1.0) and not np.any(ln1_b)),
        ln2=bool(np.all(ln2_w == 1.0) and not np.any(ln2_b)),
    )
    fast = all(trivial.values())

    d1t = np.zeros((NCORES, P, BLOCKS_PER_CORE), np.float32)
    epsdeg = np.zeros((NCORES, P, BLOCKS_PER_CORE), np.float32)
    lbt = np.zeros((NCORES, P, BLOCKS_PER_CORE), np.float16)
    for c in range(NCORES):
        sl = slice(c * NODES_PER_CORE, (c + 1) * NODES_PER_CORE)
        d1 = 1.0 / np.sqrt(degpos[sl])
        d1t[c] = d1.reshape(BLOCKS_PER_CORE, P).T
        ed = (LN_EPS * degpos[sl]) if fast else np.full(
            NODES_PER_CORE, LN_EPS, np.float32)
        epsdeg[c] = ed.reshape(BLOCKS_PER_CORE, P).T
        lb = (batpos[sl] - g_base[c]).astype(np.float16)
        lbt[c] = lb.reshape(BLOCKS_PER_CORE, P).T.astype(np.float16)

    # ---- edges -> cells (dst block x src residue class), padded to K*128
    pe_src = pos[s64]
    pe_dst = pos[d64]
    blk = pe_dst >> 7
    slot_s = pe_src & 127
    res = (slot_s >> 5).astype(np.int64)
    idx_g = ((pe_src >> 7) * 32 + (slot_s & 31)).astype(np.int64)  # global row
    ld = (pe_dst & 127).astype(np.float16)
    cell = blk * RES + res
    counts = np.bincount(cell, minlength=NBLK * RES)
    K = int(np.ceil(counts.max() / P))
    CELL = K * P

    order = np.argsort(cell, kind="stable")
    starts = np.cumsum(counts) - counts
    within = np.arange(E) - np.repeat(starts, counts)
    flat = cell[order] * CELL + within
    # pad slots gather a zero table row (a pad node) of the right residue
    apos = np.arange(NPAD)
    padrow = np.zeros(RES, np.int64)
    for rr_ in range(RES):
        cand = np.nonzero((((apos & 127) >> 5) == rr_) & (node_at < 0))[0]
        pp = int(cand[0])
        padrow[rr_] = (pp >> 7) * 32 + (pp & 31)
    idxA = np.tile(np.repeat(padrow, CELL), NBLK)
    ldA = np.full(NBLK * RES * CELL, -1.0, np.float16)
    idxA[flat] = idx_g[order]
    ldA[flat] = ld[order]
    idxA = idxA.reshape(NBLK, RES, CELL)
    ldA = ldA.reshape(NBLK, RES, K, P)

    call_len = B_GRP * CELL
    idxw1 = np.zeros((NCORES, N_GRP * RES, P, call_len // 16), np.int16)
    idxw2 = np.zeros((NCORES, N_GRP * RES, P, call_len // 16), np.int16)
    ldt = np.zeros((NCORES, P, BLOCKS_PER_CORE * RES * K), np.float16)
    for c in range(NCORES):
        b0 = c * BLOCKS_PER_CORE
        idxc = (idxA - 3200 * c) % (RES * CL_PC * 2)  # mod 25600, rotated
        for g in range(N_GRP):
            for rr in range(RES):
                l1 = idxc[b0 + g * B_GRP: b0 + (g + 1) * B_GRP, rr, :].reshape(-1)
                l2 = idxA[b0 + g * B_GRP: b0 + (g + 1) * B_GRP, rr, :].reshape(-1)
                w1 = l1.astype(np.int16).reshape(-1, 16).T
                w2 = l2.astype(np.int16).reshape(-1, 16).T
                idxw1[c, g * RES + rr] = np.tile(w1, (8, 1))
                idxw2[c, g * RES + rr] = np.tile(w2, (8, 1))
        # ldt[c][p, b*RES*K + rr*K + k] = ldA[b0+b, rr, k, p]
        ldt[c] = ldA[b0:b0 + BLOCKS_PER_CORE].reshape(
            BLOCKS_PER_CORE * RES * K, P).T

    # ---- classifier / epilogue host data
    WnT = (cls_g[:, None] * cls_v
           / np.linalg.norm(cls_v, axis=1, keepdims=True)).T.astype(np.float32)
    cnt = np.maximum(np.bincount(batch, minlength=G).astype(np.float32), 1.0)

    return dict(
        K=K, xsT=xsT, d1t=d1t, epsdeg=epsdeg, lbt=lbt,
        idxw1=idxw1, idxw2=idxw2, ldt=ldt,
        WnT=WnT, cnt=cnt, g_base=g_base, trivial=trivial,
        W1h=W1.astype(np.float16), W2h=W2.astype(np.float16),
        b1=b1.astype(np.float32), b2=b2.astype(np.float32),
        ln1_w=ln1_w.astype(np.float32), ln1_b=ln1_b.astype(np.float32),
        ln2_w=ln2_w.astype(np.float32), ln2_b=ln2_b.astype(np.float32),
        cls_b=cls_b.astype(np.float32),
    )


# ---------------------------------------------------------------- program
def _build(K: int, trivial: dict):
    CELL = K * P
    call_len = B_GRP * CELL
    CW = call_len // 16
    RK = RES * K
    GH = B_GRP * H
    fast = all(trivial.values())

    nc = bacc.Bacc(None, target_bir_lowering=False, debug=False,
                   num_devices=NCORES, num_swdge_queues=4)

    xsT_p = nc.declare_dram_parameter("xsT", [F, NPAD], F16, isOutput=False)
    W1_p = nc.declare_dram_parameter("W1h", [F, H], F16, isOutput=False)
    W2_p = nc.declare_dram_parameter("W2h", [H, H], F16, isOutput=False)
    idxw1_p = nc.declare_dram_parameter(
        "idxw1", [N_GRP * RES, P, CW], I16, isOutput=False)
    idxw2_p = nc.declare_dram_parameter(
        "idxw2", [N_GRP * RES, P, CW], I16, isOutput=False)
    ldt_p = nc.declare_dram_parameter(
        "ldt", [P, BLOCKS_PER_CORE * RK], F16, isOutput=False)
    epsdeg_p = nc.declare_dram_parameter(
        "epsdeg", [P, BLOCKS_PER_CORE], F32, isOutput=False)
    d1t_p = nc.declare_dram_parameter("d1t", [P, BLOCKS_PER_CORE], F32, isOutput=False)
    lbt_p = nc.declare_dram_parameter("lbt", [P, BLOCKS_PER_CORE], F16, isOutput=False)
    WnT_p = nc.declare_dram_parameter("WnT", [H, C], F32, isOutput=False)
    rows_p = {}
    for nm in ["b1r", "b2r", "ln1wr", "ln1br", "ln2wr", "ln2br"]:
        rows_p[nm] = nc.declare_dram_parameter(nm, [1, H], F32, isOutput=False)
    out_p = nc.declare_dram_parameter("out_part", [P, C], F32, isOutput=True)

    with tile.TileContext(nc, num_cores=NCORES) as tc:
        with (
            tc.tile_pool(name="consts", bufs=1) as consts,
            tc.tile_pool(name="resident", bufs=1) as resident,
            tc.tile_pool(name="work", bufs=3) as work,
            tc.tile_pool(name="gat", bufs=2) as gatp,
            tc.tile_pool(name="spool", bufs=4) as spool,
            tc.tile_pool(name="psum_agg", bufs=4, space="PSUM") as psum_agg,
            tc.tile_pool(name="psum_pj", bufs=2, space="PSUM") as psum_pj,
            tc.tile_pool(name="psum_t", bufs=1, space="PSUM") as psum_t,
            tc.tile_pool(name="dram", bufs=1, space="DRAM") as dram,
        ):
            tables1 = [dram.tile([RES * CL_PC * 2, H], F16, tag=f"t1_{r}",
                                 name=f"t1_{r}")
                       for r in range(RES)]
            tables2 = [dram.tile([RES * CL_PC * 2, H], F16, tag=f"t2_{r}",
                                 name=f"t2_{r}", addr_space="Shared")
                       for r in range(RES)]
            ag2_in = [dram.tile([CL_PC, H], F16, tag=f"ag2i{r}",
                                name=f"ag2i{r}")
                      for r in range(RES)]

            # ---------------- constants
            W1_t = consts.tile([F, H], F16)
            nc.sync.dma_start(out=W1_t[:], in_=W1_p[:])
            W2_t = consts.tile([H, H], F16)
            nc.sync.dma_start(out=W2_t[:], in_=W2_p[:])
            epsdeg_t = consts.tile([P, BLOCKS_PER_CORE], F32)
            nc.sync.dma_start(out=epsdeg_t[:], in_=epsdeg_p[:])
            lbt_t = consts.tile([P, BLOCKS_PER_CORE], F16)
            nc.sync.dma_start(out=lbt_t[:], in_=lbt_p[:])
            WnT_t = consts.tile([H, C], F32)
            nc.sync.dma_start(out=WnT_t[:], in_=WnT_p[:])
            ldt_t = consts.tile([P, BLOCKS_PER_CORE * RK], F16)
            nc.sync.dma_start(out=ldt_t[:], in_=ldt_p[:])
            d1t_t = consts.tile([P, BLOCKS_PER_CORE], F32)
            nc.sync.dma_start(out=d1t_t[:], in_=d1t_p[:])
            rows = {}
            if not fast:
                for nm, pp in rows_p.items():
                    t = consts.tile([1, H], F32, tag=f"row_{nm}",
                                    name=f"row_{nm}")
                    nc.sync.dma_start(out=t[:], in_=pp[:])
                    rows[nm] = t

            idx1_all = consts.tile([P, N_GRP * RES * CW], I16, name="idx1_all")
            nc.sync.dma_start(out=idx1_all[:],
                              in_=idxw1_p[:].rearrange("c p w -> p c w"))
            idx2_all = consts.tile([P, N_GRP * RES * CW], I16, name="idx2_all")
            nc.sync.dma_start(out=idx2_all[:],
                              in_=idxw2_p[:].rearrange("c p w -> p c w"))

            ident_h = consts.tile([P, P], F16)
            make_identity(nc, ident_h[:])
            ident_f = consts.tile([P, P], F32)
            make_identity(nc, ident_f[:])

            # iota constants: value = position within each 128 chunk
            iota_i = consts.tile([P, RK * P], I32)
            nc.gpsimd.iota(iota_i[:], pattern=[[0, RK], [1, P]], base=0,
                           channel_multiplier=0)
            iota16 = consts.tile([P, RK * P], F16)
            nc.vector.tensor_copy(out=iota16[:], in_=iota_i[:])
            iotaP_i = consts.tile([P, B_GRP * P], I32)
            nc.gpsimd.iota(iotaP_i[:], pattern=[[0, B_GRP], [1, P]], base=0,
                           channel_multiplier=0)
            iotaP16 = consts.tile([P, B_GRP * P], F16)
            nc.vector.tensor_copy(out=iotaP16[:], in_=iotaP_i[:])

            bcos_eps_t = consts.tile([P, 1], F32)
            nc.vector.memset(bcos_eps_t[:], BCOS_EPS)

            # layer-2 self rows (own xw2), kept resident between the loops
            selfres2 = [resident.tile([P, GH], F16, tag=f"s2g{g}",
                                      name=f"s2g{g}")
                        for g in range(N_GRP)]

            t1w = [tables1[r][:].rearrange("(b q) d -> q b d", q=32)
                   for r in range(RES)]
            ag2_views = [ag2_in[r][:].rearrange("(b q) d -> q b d", q=32)
                         for r in range(RES)]

            # ---------------- proj1: replicated (dinv*x) @ W1 -> tables1
            with nc.named_scope("proj1"):
                for i in range(NBLK // PJ):
                    xt8 = work.tile([F, PJ * P], F16, tag="xt8")
                    nc.sync.dma_start(
                        out=xt8[:], in_=xsT_p[:, i * PJ * P:(i + 1) * PJ * P])
                    pp = psum_pj.tile([P, PJ * H], F32, space="PSUM", tag="pj")
                    for bl in range(PJ):
                        nc.tensor.matmul(out=pp[:, bl * H:(bl + 1) * H],
                                         lhsT=xt8[:, bl * P:(bl + 1) * P],
                                         rhs=W1_t[:], start=True, stop=True)
                    xw8 = work.tile([P, PJ * H], F16, tag="xw8")
                    if i % 2 == 0:
                        nc.scalar.activation(out=xw8[:], in_=pp[:],
                                             func=Act.Copy)
                    else:
                        nc.vector.tensor_copy(out=xw8[:], in_=pp[:])
                    for r in range(RES):
                        nc.sync.dma_start(
                            out=t1w[r][:, i * PJ:(i + 1) * PJ, :],
                            in_=xw8[:].rearrange("p (b d) -> p b d", d=H)
                            [32 * r:32 * (r + 1)])

            # ---------------- one GCN layer: gather + agg + LN + ELU
            def layer(lyr, idx_all, tabs, pool_ps):
                for g in range(N_GRP):
                    gtiles = []
                    for rr in range(RES):
                        ci = g * RES + rr
                        gt = gatp.tile([P, B_GRP * K, H], F16, tag=f"gat{rr}",
                                       bufs=2, name=f"gat{rr}")
                        nc.gpsimd.dma_gather(
                            out_ap=gt[:], in_ap=tabs[rr][:],
                            idxs_ap=idx_all[:, ci * CW:(ci + 1) * CW],
                            num_idxs=call_len,
                            num_idxs_reg=call_len, elem_size=H,
                            elem_step=H, single_packet=False,
                            queue_num=rr,
                        )
                        gtiles.append(gt)

                    # self rows: layer1 reads own table rows; layer2 has them
                    if lyr == 1:
                        selfb4 = work.tile([P, B_GRP, H], F16, tag="selfb4")
                        for r in range(RES):
                            nc.sync.dma_start(
                                out=selfb4[32 * r:32 * (r + 1)],
                                in_=t1w[r][:, g * B_GRP:(g + 1) * B_GRP, :])
                        selfv = selfb4[:].rearrange("p b d -> p (b d)")
                    else:
                        selfv = selfres2[g][:]

                    # one-hot S + aggregation matmuls, one PSUM bank per block
                    psbs = []
                    for bl in range(B_GRP):
                        lb = g * B_GRP + bl
                        Sbig = spool.tile([P, RK * P], F16, tag="Sbig",
                                          bufs=4, name="Sbig")
                        nc.vector.tensor_tensor(
                            out=Sbig[:].rearrange("p (j m) -> p j m", m=P),
                            in0=iota16[:].rearrange("p (j m) -> p j m", m=P),
                            in1=ldt_t[:, lb * RK:(lb + 1) * RK]
                            .to_broadcast([P, RK, P]),
                            op=AOp.is_equal)
                        psb = psum_agg.tile([P, H], F32, space="PSUM",
                                            tag="aggblk", name="psb")
                        for rr in range(RES):
                            for k in range(K):
                                j2 = rr * K + k
                                nc.tensor.matmul(
                                    out=psb[:],
                                    lhsT=Sbig[:, j2 * P:(j2 + 1) * P],
                                    rhs=gtiles[rr][:, bl * K + k, :],
                                    start=(j2 == 0),
                                    stop=(j2 == RK - 1),
                                )
                        psbs.append(psb)

                    # ---- fused epilogue over the 4 blocks
                    svg = work.tile([P, B_GRP], F32, tag="svg")
                    vbs = []
                    for bl in range(B_GRP):
                        vb = work.tile([P, H], F32, tag="vb", bufs=8,
                                       name="vb")
                        nc.vector.scalar_tensor_tensor(
                            out=vb[:], in0=psbs[bl][:], scalar=0.0,
                            in1=selfv[:, bl * H:(bl + 1) * H],
                            op0=AOp.add, op1=AOp.add,
                            accum_out=svg[:, bl:bl + 1])
                        vbs.append(vb)
                    if not fast:
                        b_row = rows["b1r" if lyr == 1 else "b2r"]
                        for bl in range(B_GRP):
                            lb = g * B_GRP + bl
                            nc.vector.tensor_scalar_mul(
                                out=vbs[bl][:], in0=vbs[bl][:],
                                scalar1=d1t_t[:, lb:lb + 1])
                            nc.vector.tensor_tensor(
                                out=vbs[bl][:], in0=vbs[bl][:],
                                in1=b_row[:].to_broadcast([P, H]), op=AOp.add)
                        # recompute sums after scaling
                        for bl in range(B_GRP):
                            nc.vector.tensor_reduce(
                                out=svg[:, bl:bl + 1],
                                in_=vbs[bl][:].rearrange("p (o d) -> p o d",
                                                         o=1),
                                axis=AxX, op=AOp.add)
                    mu4 = work.tile([P, B_GRP], F32, tag="mu4")
                    nc.scalar.activation(out=mu4[:], in_=svg[:],
                                         func=Act.Copy, scale=1.0 / H)
                    vs4 = work.tile([P, B_GRP], F32, tag="vs4")
                    for bl in range(B_GRP):
                        scr = work.tile([P, H], F32, tag="scr", bufs=4,
                                        name="scr")
                        nc.vector.scalar_tensor_tensor(
                            out=scr[:], in0=vbs[bl][:],
                            scalar=mu4[:, bl:bl + 1], in1=vbs[bl][:],
                            op0=AOp.subtract, op1=AOp.mult,
                            accum_out=vs4[:, bl:bl + 1])
                    vsad = work.tile([P, B_GRP], F32, tag="vsad")
                    nc.vector.scalar_tensor_tensor(
                        out=vsad[:], in0=vs4[:], scalar=1.0 / H,
                        in1=epsdeg_t[:, g * B_GRP:(g + 1) * B_GRP],
                        op0=AOp.mult, op1=AOp.add)
                    sd4 = work.tile([P, B_GRP], F32, tag="sd4")
                    nc.scalar.activation(out=sd4[:], in_=vsad[:],
                                         func=Act.Sqrt)
                    rr4 = work.tile([P, B_GRP], F32, tag="rr4")
                    nc.vector.reciprocal(out=rr4[:], in_=sd4[:])
                    vmc = work.tile([P, GH], F32, tag="vmc")
                    for bl in range(B_GRP):
                        nc.vector.tensor_scalar_sub(
                            out=vmc[:, bl * H:(bl + 1) * H], in0=vbs[bl][:],
                            scalar1=mu4[:, bl:bl + 1])
                    h4 = work.tile([P, GH], F16 if fast else F32, tag="h4")
                    nc.vector.tensor_tensor(
                        out=h4[:].rearrange("p (b d) -> p b d", d=H),
                        in0=vmc[:].rearrange("p (b d) -> p b d", d=H),
                        in1=rr4[:].to_broadcast([P, B_GRP, H]), op=AOp.mult)
                    if not fast:
                        lw = rows["ln1wr" if lyr == 1 else "ln2wr"]
                        lbr = rows["ln1br" if lyr == 1 else "ln2br"]
                        for bl in range(B_GRP):
                            nc.vector.tensor_tensor(
                                out=h4[:, bl * H:(bl + 1) * H],
                                in0=h4[:, bl * H:(bl + 1) * H],
                                in1=lw[:].to_broadcast([P, H]), op=AOp.mult)
                            nc.vector.tensor_tensor(
                                out=h4[:, bl * H:(bl + 1) * H],
                                in0=h4[:, bl * H:(bl + 1) * H],
                                in1=lbr[:].to_broadcast([P, H]), op=AOp.add)
                    # ELU(h) = min(exp(h) - 1, relu(h))
                    ex4 = work.tile([P, GH], F32, tag="ex4")
                    nc.scalar.activation(out=ex4[:], in_=h4[:], func=Act.Exp)
                    rl4 = work.tile([P, GH], F16, tag="rl4")
                    nc.vector.tensor_scalar_max(out=rl4[:], in0=h4[:],
                                                scalar1=0.0)
                    helu4 = work.tile([P, GH], F16, tag="helu4")
                    nc.vector.scalar_tensor_tensor(
                        out=helu4[:], in0=ex4[:], scalar=1.0, in1=rl4[:],
                        op0=AOp.subtract, op1=AOp.min)

                    if lyr == 1:
                        # ---- proj2 for this group: xw2 = h^T.T @ W2
                        pg = psum_t.tile([P, GH], F32, space="PSUM", tag="pg",
                                         name="pg")
                        for bl in range(B_GRP):
                            pst = psum_t.tile([P, P], F16, space="PSUM",
                                              tag="tp", name="pst")
                            nc.tensor.transpose(
                                out=pst[:], in_=helu4[:, bl * H:(bl + 1) * H],
                                identity=ident_h[:])
                            hT = work.tile([H, P], F16, tag="hT", bufs=2,
                                           name="hT")
                            nc.scalar.activation(out=hT[:], in_=pst[:],
                                                 func=Act.Copy)
                            nc.tensor.matmul(out=pg[:, bl * H:(bl + 1) * H],
                                             lhsT=hT[:], rhs=W2_t[:],
                                             start=True, stop=True)
                        # table2 rows are prescaled by dinv[src] (coef
                        # separability), matching the layer-1 host prescale
                        nc.vector.tensor_tensor(
                            out=selfres2[g][:].rearrange("p (b d) -> p b d",
                                                         d=H),
                            in0=pg[:].rearrange("p (b d) -> p b d", d=H),
                            in1=d1t_t[:, g * B_GRP:(g + 1) * B_GRP]
                            .to_broadcast([P, B_GRP, H]),
                            op=AOp.mult)
                        for r in range(RES):
                            nc.sync.dma_start(
                                out=ag2_views[r][:, g * B_GRP:(g + 1) * B_GRP, :],
                                in_=selfres2[g][:]
                                .rearrange("p (b d) -> p b d", d=H)
                                [32 * r:32 * (r + 1)])
                    else:
                        # ---- bcos residual mix + pooling
                        qs4 = work.tile([P, B_GRP], F32, tag="qs4")
                        for bl in range(B_GRP):
                            scr2 = work.tile([P, H], F32, tag="scr2", bufs=4,
                                             name="scr2")
                            nc.scalar.activation(
                                out=scr2[:], in_=helu4[:, bl * H:(bl + 1) * H],
                                func=Act.Square,
                                accum_out=qs4[:, bl:bl + 1])
                        nrm4 = work.tile([P, B_GRP], F32, tag="nrm4")
                        nc.scalar.activation(out=nrm4[:], in_=qs4[:],
                                             func=Act.Sqrt,
                                             bias=bcos_eps_t[:])
                        den4 = work.tile([P, B_GRP], F32, tag="den4")
                        nc.vector.tensor_scalar_add(out=den4[:], in0=nrm4[:],
                                                    scalar1=BCOS_EPS)
                        rcp4 = work.tile([P, B_GRP], F32, tag="rcp4")
                        nc.vector.reciprocal(out=rcp4[:], in_=den4[:])
                        fac4 = work.tile([P, B_GRP], F32, tag="fac4")
                        nc.scalar.activation(out=fac4[:], in_=rcp4[:],
                                             func=Act.Copy,
                                             scale=(1.0 - RR) * TEMP, bias=RR)
                        hb4 = work.tile([P, GH], F16, tag="hb4")
                        for bl in range(B_GRP):
                            nc.vector.tensor_scalar_mul(
                                out=hb4[:, bl * H:(bl + 1) * H],
                                in0=helu4[:, bl * H:(bl + 1) * H],
                                scalar1=fac4[:, bl:bl + 1])
                        Sp4 = spool.tile([P, B_GRP * P], F16, tag="Sp4",
                                         bufs=2, name="Sp4")
                        nc.vector.tensor_tensor(
                            out=Sp4[:].rearrange("p (b d) -> p b d", d=P),
                            in0=iotaP16[:].rearrange("p (b d) -> p b d", d=P),
                            in1=lbt_t[:, g * B_GRP:(g + 1) * B_GRP]
                            .to_broadcast([P, B_GRP, P]),
                            op=AOp.is_equal)
                        for bl in range(B_GRP):
                            lb = g * B_GRP + bl
                            nc.tensor.matmul(
                                out=pool_ps[:],
                                lhsT=Sp4[:, bl * P:(bl + 1) * P],
                                rhs=hb4[:, bl * H:(bl + 1) * H],
                                start=(lb == 0),
                                stop=(lb == BLOCKS_PER_CORE - 1))

            with nc.named_scope("layer1"):
                layer(1, idx1_all, tables1, None)

            with nc.named_scope("ag2"):
                for r in range(RES):
                    nc.gpsimd.collective_compute(
                        "AllGather", AOp.bypass,
                        replica_groups=[list(range(NCORES))],
                        ins=[ag2_in[r][:].opt()], outs=[tables2[r][:].opt()],
                    )

            pool_ps = psum_t.tile([P, H], F32, space="PSUM", tag="pg",
                                  name="pool_ps")
            with nc.named_scope("layer2"):
                layer(2, idx2_all, tables2, pool_ps)

            # ------------ pooled partial -> transpose -> classifier
            with nc.named_scope("cls"):
                pooled = work.tile([P, H], F32, tag="pooled")
                nc.vector.tensor_copy(out=pooled[:], in_=pool_ps[:])
                psT = psum_t.tile([P, P], F32, space="PSUM", tag="tp",
                                  name="psT")
                nc.tensor.transpose(out=psT[:], in_=pooled[:],
                                    identity=ident_f[:])
                pooledT = work.tile([P, P], F32, tag="pooledT")
                nc.vector.tensor_copy(out=pooledT[:], in_=psT[:])
                cls_ps = psum_t.tile([P, C], F32, space="PSUM", tag="pg",
                                     name="cls_ps")
                nc.tensor.matmul(out=cls_ps[:], lhsT=pooledT[:], rhs=WnT_t[:],
                                 start=True, stop=True)
                outt = work.tile([P, C], F32, tag="outt")
                nc.vector.tensor_copy(out=outt[:], in_=cls_ps[:])
                nc.sync.dma_start(out=out_p[:], in_=outt[:])

    nc.finalize()
    return nc


_CACHE: dict = {}
LAST_RESULTS = None


def _ensure_ntff_hook():
    """Install the antenv.axon_hooks shim so trace=True captures NTFF
    profiles through the axon PJRT .so (the trimmed container lacks the
    module trn_boot expects)."""
    import sys as _sys
    import types

    if "antenv.axon_hooks" not in _sys.modules:
        mod = types.ModuleType("antenv.axon_hooks")
        holder = [None]
        mod.set_axon_ntff_profile_hook = lambda h: holder.__setitem__(0, h)
        mod.get_axon_ntff_profile_hook = lambda: holder[0]
        _sys.modules["antenv.axon_hooks"] = mod
        import antenv

        antenv.axon_hooks = mod
    from antenv.axon_hooks import (get_axon_ntff_profile_hook,
                                   set_axon_ntff_profile_hook)

    if get_axon_ntff_profile_hook() is None:
        from trn_agent_boot.trn_boot import _ntff_profile_via_ctypes

        h = _ntff_profile_via_ctypes("/opt/axon/libaxon_pjrt.so")
        if h is not None:
            set_axon_ntff_profile_hook(h)


def kernel(**inputs) -> np.ndarray:
    np_inputs = {k: np.asarray(v) for k, v in inputs.items()}
    prep = _prep(**np_inputs)
    K = prep["K"]
    tkey = (K, tuple(sorted(prep["trivial"].items())))
    if tkey not in _CACHE:
        _CACHE[tkey] = _build(K, prep["trivial"])
    nc = _CACHE[tkey]

    in_maps = []
    for c in range(NCORES):
        in_maps.append(dict(
            xsT=prep["xsT"][c], W1h=prep["W1h"], W2h=prep["W2h"],
            idxw1=prep["idxw1"][c], idxw2=prep["idxw2"][c],
            ldt=prep["ldt"][c], epsdeg=prep["epsdeg"][c],
            d1t=prep["d1t"][c], lbt=prep["lbt"][c], WnT=prep["WnT"],
            b1r=prep["b1"][None, :], b2r=prep["b2"][None, :],
            ln1wr=prep["ln1_w"][None, :], ln1br=prep["ln1_b"][None, :],
            ln2wr=prep["ln2_w"][None, :], ln2br=prep["ln2_b"][None, :],
        ))
    import os
    trace = bool(os.environ.get("BASS_KERNEL_TRACE"))
    if trace:
        _ensure_ntff_hook()
    res = run_bass_kernel_spmd(nc, in_maps, core_ids=list(range(NCORES)),
                               trace=trace)
    global LAST_RESULTS
    LAST_RESULTS = res
    if trace and res.exec_time_ns is not None:
        print(f"HW exec time: {res.exec_time_ns} ns", flush=True)

    # host unshard: scatter-add partial logits by per-core graph base,
    # divide by graph node counts, add classifier bias
    out = np.zeros((G, C), np.float64)
    for c in range(NCORES):
        part = res.results[c]["out_part"].astype(np.float64)
        gb = int(prep["g_base"][c])
        hi = min(G, gb + P)
        out[gb:hi] += part[: hi - gb]
    out = out / prep["cnt"][:, None] + prep["cls_b"][None, :]
    return out.astype(np.float32)


# revision 17
# speedup vs baseline: 1.3649x; 1.3649x over previous
"""Trainium2 Bass kernel for a 2-layer BCos-GCN (nn_BCosGCN_28346784153649).

Strategy (8 NeuronCores, SPMD):
  - Nodes (and their incident edges, grouped by destination block) are
    sharded across the 8 cores; the 128x128 weights are replicated.
  - Tables are ONE flat fp16 [102400, 128] tensor per layer; the 4 "residue
    banks" (slot % 4) are stride-4 row views of it, so int16 gather indices
    stay < 25600 while projection writes / self-row reads are single big
    DMAs.
  - Layer 1: the projection (dinv*x) @ W1 is REPLICATED on every core from a
    host-staged fp16 x^T (rotated per core so each core's own nodes occupy
    flat-table rows [0, 12800)) -- no collective needed.
  - Aggregation per layer: dma_gather source rows by residue bank and
    accumulate per 128-destination-node block via one-hot matmuls (PSUM
    accumulation).  The one-hot S matrices are built ON-CHIP with one
    is_equal per 4-block group against an iota constant.
  - LayerNorm scale-invariance: b1/b2 are zero, so the dinv[dst] scaling
    before LN is folded away exactly via a per-node H*eps*deg bias added to
    the variance sum.
  - Layer 2 projection runs inside the layer-1 loop per 4-block group; ONE
    AllGather (Shared output) assembles the rank-major layer-2 table; the
    layer-2 self rows stay resident in SBUF.
  - Global mean-pool via one-hot matmul accumulation + weight-normalized
    classifier; tiny [128, 10] per-core partials are combined on the host.
"""

import sys

sys.path.insert(0, "/opt/trn_rl_repo")

import numpy as np

from concourse import bacc, tile, mybir
from concourse.bass_utils import run_bass_kernel_spmd
from concourse.masks import make_identity

# ---------------------------------------------------------------- constants
N, E, F, H, C, G = 100000, 1600000, 128, 128, 10, 512
LN_EPS = 1e-5
BCOS_EPS = 1e-6
TEMP = 1.5
RR = 0.6  # residual ratio; bcos exponent B == 1.0 -> bcos(h) = TEMP*h/(nrm+eps)

NCORES = 8
P = 128
REAL_PER_CORE = N // NCORES            # 12500
NODES_PER_CORE = 12800                 # padded: 100 blocks of 128
BLOCKS_PER_CORE = NODES_PER_CORE // P  # 100
NPAD = NODES_PER_CORE * NCORES         # 102400
NBLK = NPAD // P                       # 800
RES = 4                                # residue banks (slot % 4)
B_GRP = 4                              # dst blocks per gather call / group
N_GRP = BLOCKS_PER_CORE // B_GRP       # 25 groups per core
ROWS_RES = NPAD // RES                 # 25600 rows per residue view
PJ = 4                                 # blocks per proj1 iteration

F16 = mybir.dt.float16
F32 = mybir.dt.float32
I16 = mybir.dt.int16
I32 = mybir.dt.int32
AOp = mybir.AluOpType
Act = mybir.ActivationFunctionType
AxX = mybir.AxisListType.X


# ---------------------------------------------------------------- host prep
def _lpt_blocks(indeg_core: np.ndarray) -> list[list[int]]:
    """Pack the core's real nodes into 100 blocks of <=128, balancing the
    in-degree sum per block (greedy LPT)."""
    import heapq

    order = np.argsort(-indeg_core, kind="stable")
    heap = [(0, 0, b) for b in range(BLOCKS_PER_CORE)]
    heapq.heapify(heap)
    blocks: list[list[int]] = [[] for _ in range(BLOCKS_PER_CORE)]
    for v in order:
        while True:
            load, cnt, b = heapq.heappop(heap)
            if cnt < P:
                break
        blocks[b].append(int(v))
        heapq.heappush(heap, (load + int(indeg_core[v]), cnt + 1, b))
    return blocks


def _color_banks(ownblk, src, dstblk, rounds=24, seed=0):
    """Greedy residue-bank coloring balancing (dst-block, color) edge cells
    at <=512 (-> K=4), subject to <=32 nodes per (own-block, color)."""
    SLOT_CAP = P // RES
    Nn = ownblk.shape[0]
    rng = np.random.default_rng(seed)
    eorder = np.argsort(src, kind="stable")
    e_dstblk = dstblk[eorder]
    esrc = src[eorder]
    degn = np.bincount(src, minlength=Nn)
    estart = np.concatenate([[0], np.cumsum(degn)])
    cellcnt = np.zeros((NBLK, RES), np.int64)
    slotcnt = np.zeros((NBLK, RES), np.int32)
    color = np.full(Nn, -1, np.int32)
    order = np.argsort(-degn, kind="stable")
    target = max(1.0, dstblk.shape[0] / (NBLK * RES))
    cap = int(np.ceil(target / P) * P)
    for bt in np.array_split(order, rounds):
        nb = bt.shape[0]
        reps = degn[bt]
        node_rep = np.repeat(np.arange(nb), reps)
        eidx = (np.concatenate([np.arange(estart[v], estart[v + 1]) for v in bt])
                if nb else np.empty(0, np.int64))
        score = np.zeros((nb, RES), np.float64)
        if eidx.size:
            np.add.at(score, node_rep, cellcnt[e_dstblk[eidx]])
        own = ownblk[bt]
        score += np.where(slotcnt[own] >= SLOT_CAP, 1e12, 0.0)
        if eidx.size:
            np.add.at(score, node_rep,
                      np.where(cellcnt[e_dstblk[eidx]] >= cap - 1, 1e6, 0.0))
        score += rng.random((nb, RES))
        ch = np.argmin(score, axis=1).astype(np.int32)
        for i in range(nb):
            o, c = own[i], ch[i]
            if slotcnt[o, c] >= SLOT_CAP:
                c = int(np.argmin(slotcnt[o] + np.where(
                    slotcnt[o] >= SLOT_CAP, 10**9, 0)))
                ch[i] = c
            slotcnt[o, c] += 1
        color[bt] = ch
        if eidx.size:
            np.add.at(cellcnt, (e_dstblk[eidx], ch[node_rep]), 1)
    # exact repair: move nodes out of over-cap cells
    border = np.argsort(e_dstblk, kind="stable")
    bcnt = np.bincount(e_dstblk, minlength=NBLK)
    bstart = np.concatenate([[0], np.cumsum(bcnt)])
    for _ in range(40):
        over = np.argwhere(cellcnt > cap)
        if over.size == 0:
            break
        for bb, cc in over:
            while cellcnt[bb, cc] > cap:
                cands = np.unique(esrc[border[bstart[bb]:bstart[bb + 1]]])
                cands = cands[color[cands] == cc]
                moved = False
                contrib = np.array([
                    np.count_nonzero(e_dstblk[estart[v]:estart[v + 1]] == bb)
                    for v in cands])
                for v in cands[np.argsort(contrib)]:
                    o = ownblk[v]
                    blks = e_dstblk[estart[v]:estart[v + 1]]
                    for c2 in np.argsort(cellcnt[bb]):
                        if c2 == cc or slotcnt[o, c2] >= SLOT_CAP:
                            continue
                        add = np.bincount(blks, minlength=NBLK)
                        touched = np.nonzero(add)[0]
                        if (cellcnt[touched, c2] + add[touched] <= cap).all():
                            cellcnt[touched, cc] -= add[touched]
                            cellcnt[touched, c2] += add[touched]
                            slotcnt[o, cc] -= 1
                            slotcnt[o, c2] += 1
                            color[v] = c2
                            moved = True
                            break
                    if moved:
                        break
                if not moved:
                    break
    return color


def _prep(x, src, dst, batch, W1, b1, ln1_w, ln1_b, W2, b2, ln2_w, ln2_b,
          cls_v, cls_g, cls_b, seed=0):
    indeg = np.bincount(dst, minlength=N)
    deg = indeg.astype(np.float32) + 1.0
    dinv = (1.0 / np.sqrt(deg)).astype(np.float32)

    # ---- node -> (core, block); LPT balance in-degree per block
    ownblk = np.zeros(N, np.int64)
    core_blocks = []
    g_base = np.zeros(NCORES, np.int64)
    for c in range(NCORES):
        lo, hi = c * REAL_PER_CORE, (c + 1) * REAL_PER_CORE
        g_base[c] = int(batch[lo])
        span = int(batch[hi - 1]) - g_base[c]
        assert span < P, f"core {c} spans {span + 1} graphs > 128"
        blocks = _lpt_blocks(indeg[lo:hi])
        core_blocks.append(blocks)
        for b in range(BLOCKS_PER_CORE):
            for v_local in blocks[b]:
                ownblk[lo + v_local] = c * BLOCKS_PER_CORE + b

    # ---- residue-bank coloring (cells <= 512 -> K=4); slot assignment
    # color r occupies slots {r, r+4, ...}: residue class = slot % 4, so the
    # 4 residue banks are stride-4 row views of ONE flat table.
    s64 = src.astype(np.int64)
    d64 = dst.astype(np.int64)
    color = _color_banks(ownblk, s64, ownblk[d64])
    pos = np.full(N, -1, np.int64)
    for c in range(NCORES):
        lo = c * REAL_PER_CORE
        for b in range(BLOCKS_PER_CORE):
            blk = core_blocks[c][b]
            base = c * NODES_PER_CORE + b * P
            nxt = [0, 0, 0, 0]
            for v_local in blk:
                cc = int(color[lo + v_local])
                sl = cc + RES * nxt[cc]
                nxt[cc] += 1
                pos[lo + v_local] = base + sl
    assert (pos >= 0).all()

    # ---- per-position node data (pad positions keep zeros / neutral values)
    node_at = np.full(NPAD, -1, np.int64)
    node_at[pos] = np.arange(N)
    ok = node_at >= 0
    sel = node_at[ok]

    # x^T, prescaled by dinv; per-core rotated copies (own nodes first)
    xpos = np.zeros((NPAD, F), np.float32)
    xpos[ok] = x[sel] * dinv[sel][:, None]
    xsTg = np.ascontiguousarray(xpos.T.astype(np.float16))       # [F, NPAD]
    xsT = [np.ascontiguousarray(np.roll(xsTg, -NODES_PER_CORE * c, axis=1))
           for c in range(NCORES)]

    degpos = np.ones(NPAD, np.float32)
    degpos[ok] = deg[sel]
    batpos = np.zeros(NPAD, np.float32)
    batpos[ok] = batch[sel].astype(np.float32)

    trivial = dict(
        b1=not np.any(b1), b2=not np.any(b2),
        ln1=bool(np.all(ln1_w == 1.0) and not np.any(ln1_b)),
        ln2=bool(np.all(ln2_w == 1.0) and not np.any(ln2_b)),
    )
    fast = all(trivial.values())

    d1t = np.zeros((NCORES, P, BLOCKS_PER_CORE), np.float32)
    epsdegH = np.zeros((NCORES, P, BLOCKS_PER_CORE), np.float32)
    lbt = np.zeros((NCORES, P, BLOCKS_PER_CORE), np.float16)
    for c in range(NCORES):
        sl = slice(c * NODES_PER_CORE, (c + 1) * NODES_PER_CORE)
        d1 = 1.0 / np.sqrt(degpos[sl])
        d1t[c] = d1.reshape(BLOCKS_PER_CORE, P).T
        # variance correction: sqrt((vs + H*eps*deg)/H) == sqrt(vs/H + eps*deg)
        ed = (H * LN_EPS * degpos[sl]) if fast else np.full(
            NODES_PER_CORE, H * LN_EPS, np.float32)
        epsdegH[c] = ed.reshape(BLOCKS_PER_CORE, P).T
        lb = (batpos[sl] - g_base[c]).astype(np.float16)
        lbt[c] = lb.reshape(BLOCKS_PER_CORE, P).T.astype(np.float16)

    # ---- edges -> cells (dst block x src residue class), padded to K*128
    pe_src = pos[s64]
    pe_dst = pos[d64]
    blk = pe_dst >> 7
    slot_s = pe_src & 127
    res = slot_s % RES
    idx_g = ((pe_src >> 7) * 32 + (slot_s >> 2)).astype(np.int64)  # view row
    ld = (pe_dst & 127).astype(np.float16)
    cell = blk * RES + res
    counts = np.bincount(cell, minlength=NBLK * RES)
    K = int(np.ceil(counts.max() / P))
    CELL = K * P

    order = np.argsort(cell, kind="stable")
    starts = np.cumsum(counts) - counts
    within = np.arange(E) - np.repeat(starts, counts)
    flat = cell[order] * CELL + within
    # pad slots gather a zero table row (a pad node) of the right residue
    apos = np.arange(NPAD)
    padrow = np.zeros(RES, np.int64)
    for rr_ in range(RES):
        cand = np.nonzero((((apos & 127) % RES) == rr_) & (node_at < 0))[0]
        pp = int(cand[0])
        padrow[rr_] = (pp >> 7) * 32 + ((pp & 127) >> 2)
    idxA = np.tile(np.repeat(padrow, CELL), NBLK)
    ldA = np.full(NBLK * RES * CELL, -1.0, np.float16)
    idxA[flat] = idx_g[order]
    ldA[flat] = ld[order]
    idxA = idxA.reshape(NBLK, RES, CELL)
    ldA = ldA.reshape(NBLK, RES, K, P)

    call_len = B_GRP * CELL
    idxw1 = np.zeros((NCORES, N_GRP * RES, P, call_len // 16), np.int16)
    idxw2 = np.zeros((NCORES, N_GRP * RES, P, call_len // 16), np.int16)
    ldt = np.zeros((NCORES, P, BLOCKS_PER_CORE * RES * K), np.float16)
    for c in range(NCORES):
        b0 = c * BLOCKS_PER_CORE
        idxc = (idxA - 3200 * c) % ROWS_RES  # rotated view rows
        for g in range(N_GRP):
            for rr in range(RES):
                l1 = idxc[b0 + g * B_GRP: b0 + (g + 1) * B_GRP, rr, :].reshape(-1)
                l2 = idxA[b0 + g * B_GRP: b0 + (g + 1) * B_GRP, rr, :].reshape(-1)
                w1 = l1.astype(np.int16).reshape(-1, 16).T
                w2 = l2.astype(np.int16).reshape(-1, 16).T
                idxw1[c, g * RES + rr] = np.tile(w1, (8, 1))
                idxw2[c, g * RES + rr] = np.tile(w2, (8, 1))
        # ldt[c][p, b*RES*K + rr*K + k] = ldA[b0+b, rr, k, p]
        ldt[c] = ldA[b0:b0 + BLOCKS_PER_CORE].reshape(
            BLOCKS_PER_CORE * RES * K, P).T

    # ---- classifier / epilogue host data
    WnT = (cls_g[:, None] * cls_v
           / np.linalg.norm(cls_v, axis=1, keepdims=True)).T.astype(np.float32)
    cnt = np.maximum(np.bincount(batch, minlength=G).astype(np.float32), 1.0)

    iota16 = np.tile(np.arange(P, dtype=np.float16),
                     (P, B_GRP * RES * K)).reshape(P, -1)
    iotaP16 = np.tile(np.arange(P, dtype=np.float16),
                      (P, B_GRP)).reshape(P, -1)

    return dict(
        K=K, xsT=xsT, d1t=d1t, epsdegH=epsdegH, lbt=lbt,
        idxw1=idxw1, idxw2=idxw2, ldt=ldt, iota16=iota16, iotaP16=iotaP16,
        WnT=WnT, cnt=cnt, g_base=g_base, trivial=trivial,
        W1h=W1.astype(np.float16), W2h=W2.astype(np.float16),
        b1=b1.astype(np.float32), b2=b2.astype(np.float32),
        ln1_w=ln1_w.astype(np.float32), ln1_b=ln1_b.astype(np.float32),
        ln2_w=ln2_w.astype(np.float32), ln2_b=ln2_b.astype(np.float32),
        cls_b=cls_b.astype(np.float32),
    )


# ---------------------------------------------------------------- program
def _build(K: int, trivial: dict):
    CELL = K * P
    call_len = B_GRP * CELL
    CW = call_len // 16
    RK = RES * K
    GH = B_GRP * H
    GRK = B_GRP * RK
    fast = all(trivial.values())

    nc = bacc.Bacc(None, target_bir_lowering=False, debug=False,
                   num_devices=NCORES, num_swdge_queues=4)

    xsT_p = nc.declare_dram_parameter("xsT", [F, NPAD], F16, isOutput=False)
    W1_p = nc.declare_dram_parameter("W1h", [F, H], F16, isOutput=False)
    W2_p = nc.declare_dram_parameter("W2h", [H, H], F16, isOutput=False)
    idxw1_p = nc.declare_dram_parameter(
        "idxw1", [N_GRP * RES, P, CW], I16, isOutput=False)
    idxw2_p = nc.declare_dram_parameter(
        "idxw2", [N_GRP * RES, P, CW], I16, isOutput=False)
    ldt_p = nc.declare_dram_parameter(
        "ldt", [P, BLOCKS_PER_CORE * RK], F16, isOutput=False)
    epsdegH_p = nc.declare_dram_parameter(
        "epsdegH", [P, BLOCKS_PER_CORE], F32, isOutput=False)
    d1t_p = nc.declare_dram_parameter("d1t", [P, BLOCKS_PER_CORE], F32, isOutput=False)
    lbt_p = nc.declare_dram_parameter("lbt", [P, BLOCKS_PER_CORE], F16, isOutput=False)
    WnT_p = nc.declare_dram_parameter("WnT", [H, C], F32, isOutput=False)
    iota16_p = nc.declare_dram_parameter(
        "iota16", [P, B_GRP * RES * K * P], F16, isOutput=False)
    iotaP16_p = nc.declare_dram_parameter(
        "iotaP16", [P, B_GRP * P], F16, isOutput=False)
    rows_p = {}
    for nm in ["b1r", "b2r", "ln1wr", "ln1br", "ln2wr", "ln2br"]:
        rows_p[nm] = nc.declare_dram_parameter(nm, [1, H], F32, isOutput=False)
    out_p = nc.declare_dram_parameter("out_part", [P, C], F32, isOutput=True)

    with tile.TileContext(nc, num_cores=NCORES) as tc:
        with (
            tc.tile_pool(name="consts", bufs=1) as consts,
            tc.tile_pool(name="resident", bufs=1) as resident,
            tc.tile_pool(name="work", bufs=3) as work,
            tc.tile_pool(name="gat", bufs=2) as gatp,
            tc.tile_pool(name="spool", bufs=2) as spool,
            tc.tile_pool(name="psum_agg", bufs=3, space="PSUM") as psum_agg,
            tc.tile_pool(name="psum_pj", bufs=3, space="PSUM") as psum_pj,
            tc.tile_pool(name="psum_t", bufs=1, space="PSUM") as psum_t,
            tc.tile_pool(name="dram", bufs=1, space="DRAM") as dram,
        ):
            tables1 = dram.tile([NPAD, H], F16, tag="t1", name="t1")
            tables2 = dram.tile([NPAD, H], F16, tag="t2", name="t2",
                                addr_space="Shared")
            ag2_in = dram.tile([NODES_PER_CORE, H], F16, tag="ag2i",
                               name="ag2i")

            # stride-4 residue-bank row views for the gathers
            def res_views(tab):
                quad = tab[:].rearrange("(n four) d -> n (four d)", four=RES)
                return [quad[:, r * H:(r + 1) * H] for r in range(RES)]

            t1res = res_views(tables1)
            t2res = res_views(tables2)

            # ---------------- constants
            W1_t = consts.tile([F, H], F16)
            nc.sync.dma_start(out=W1_t[:], in_=W1_p[:])
            W2_t = consts.tile([H, H], F16)
            nc.sync.dma_start(out=W2_t[:], in_=W2_p[:])
            epsdegH_t = consts.tile([P, BLOCKS_PER_CORE], F32)
            nc.sync.dma_start(out=epsdegH_t[:], in_=epsdegH_p[:])
            lbt_t = consts.tile([P, BLOCKS_PER_CORE], F16)
            nc.sync.dma_start(out=lbt_t[:], in_=lbt_p[:])
            WnT_t = consts.tile([H, C], F32)
            nc.sync.dma_start(out=WnT_t[:], in_=WnT_p[:])
            ldt_t = consts.tile([P, BLOCKS_PER_CORE * RK], F16)
            nc.sync.dma_start(out=ldt_t[:], in_=ldt_p[:])
            d1t_t = consts.tile([P, BLOCKS_PER_CORE], F32)
            nc.sync.dma_start(out=d1t_t[:], in_=d1t_p[:])
            rows = {}
            if not fast:
                for nm, pp in rows_p.items():
                    t = consts.tile([1, H], F32, tag=f"row_{nm}",
                                    name=f"row_{nm}")
                    nc.sync.dma_start(out=t[:], in_=pp[:])
                    rows[nm] = t

            # both gather index sets share one SBUF slot; the idx2 load waits
            # (via WAR) until the last layer-1 gather has issued
            idx1_all = consts.tile([P, N_GRP * RES * CW], I16, tag="idx",
                                   name="idx1_all")
            nc.sync.dma_start(out=idx1_all[:],
                              in_=idxw1_p[:].rearrange("c p w -> p c w"))

            ident_h = consts.tile([P, P], F16)
            make_identity(nc, ident_h[:])
            ident_f = consts.tile([P, P], F32)
            make_identity(nc, ident_f[:])

            # iota constants: value = position within each 128 chunk
            iota16 = consts.tile([P, GRK * P], F16)
            nc.sync.dma_start(out=iota16[:], in_=iota16_p[:])
            iotaP16 = consts.tile([P, B_GRP * P], F16)
            nc.sync.dma_start(out=iotaP16[:], in_=iotaP16_p[:])

            bcos_eps_t = consts.tile([P, 1], F32)
            nc.vector.memset(bcos_eps_t[:], BCOS_EPS)

            # layer-2 self rows (own xw2), kept resident between the loops
            selfres2 = [resident.tile([P, GH], F16, tag=f"s2g{g}",
                                      name=f"s2g{g}")
                        for g in range(N_GRP)]

            # ---------------- proj1: replicated (dinv*x) @ W1 -> tables1
            with nc.named_scope("proj1"):
                for i in range(NBLK // PJ):
                    xt4 = work.tile([F, PJ * P], F16, tag="xt4")
                    nc.sync.dma_start(
                        out=xt4[:], in_=xsT_p[:, i * PJ * P:(i + 1) * PJ * P])
                    pp = psum_pj.tile([P, PJ * H], F32, space="PSUM", tag="pj")
                    for bl in range(PJ):
                        nc.tensor.matmul(out=pp[:, bl * H:(bl + 1) * H],
                                         lhsT=xt4[:, bl * P:(bl + 1) * P],
                                         rhs=W1_t[:], start=True, stop=True)
                    xw4 = work.tile([P, PJ * H], F16, tag="xw4")
                    if i % 2 == 0:
                        nc.scalar.activation(out=xw4[:], in_=pp[:],
                                             func=Act.Copy)
                    else:
                        nc.vector.tensor_copy(out=xw4[:], in_=pp[:])
                    nc.sync.dma_start(
                        out=tables1[i * PJ * P:(i + 1) * PJ * P]
                        .rearrange("(b p) d -> p b d", p=P),
                        in_=xw4[:].rearrange("p (b d) -> p b d", d=H))

            # ---------------- one GCN layer: gather + agg + LN + ELU
            def layer(lyr, idx_all, tabs, pool_ps):
                for g in range(N_GRP):
                    gtiles = []
                    for rr in range(RES):
                        ci = g * RES + rr
                        gt = gatp.tile([P, B_GRP * K, H], F16, tag=f"gat{rr}",
                                       bufs=2, name=f"gat{rr}")
                        nc.gpsimd.dma_gather(
                            out_ap=gt[:], in_ap=tabs[rr],
                            idxs_ap=idx_all[:, ci * CW:(ci + 1) * CW],
                            num_idxs=call_len,
                            num_idxs_reg=call_len, elem_size=H,
                            elem_step=RES * H, single_packet=False,
                            queue_num=rr,
                        )
                        gtiles.append(gt)

                    # self rows: layer1 reads own table rows; layer2 has them
                    if lyr == 1:
                        selfb4 = work.tile([P, B_GRP, H], F16, tag="selfb4")
                        nc.sync.dma_start(
                            out=selfb4[:],
                            in_=tables1[g * GH:(g + 1) * GH]
                            .rearrange("(b p) d -> p b d", p=P))
                        selfv = selfb4[:].rearrange("p b d -> p (b d)")
                    else:
                        selfv = selfres2[g][:]

                    # one-hot S for the whole group with a single is_equal
                    Sbig = spool.tile([P, GRK * P], F16, tag="Sbig",
                                      bufs=2, name="Sbig")
                    nc.vector.tensor_tensor(
                        out=Sbig[:].rearrange("p (j m) -> p j m", m=P),
                        in0=iota16[:].rearrange("p (j m) -> p j m", m=P),
                        in1=ldt_t[:, g * GRK:(g + 1) * GRK]
                        .to_broadcast([P, GRK, P]),
                        op=AOp.is_equal)

                    # aggregation matmuls: one PSUM bank per group
                    ps4 = psum_agg.tile([P, GH], F32, space="PSUM",
                                        tag="agg4", name="ps4")
                    for bl in range(B_GRP):
                        for rr in range(RES):
                            for k in range(K):
                                j2 = rr * K + k
                                nc.tensor.matmul(
                                    out=ps4[:, bl * H:(bl + 1) * H],
                                    lhsT=Sbig[:, (bl * RK + j2) * P:
                                              (bl * RK + j2 + 1) * P],
                                    rhs=gtiles[rr][:, bl * K + k, :],
                                    start=(j2 == 0),
                                    stop=(j2 == RK - 1),
                                )

                    # ---- epilogue over the 4 blocks [P, 4H]
                    v4 = work.tile([P, GH], F32, tag="v4")
                    nc.vector.tensor_tensor(out=v4[:], in0=ps4[:], in1=selfv,
                                            op=AOp.add)
                    if not fast:
                        b_row = rows["b1r" if lyr == 1 else "b2r"]
                        nc.vector.tensor_tensor(
                            out=v4[:].rearrange("p (b d) -> p b d", d=H),
                            in0=v4[:].rearrange("p (b d) -> p b d", d=H),
                            in1=d1t_t[:, g * B_GRP:(g + 1) * B_GRP]
                            .to_broadcast([P, B_GRP, H]), op=AOp.mult)
                        nc.vector.tensor_tensor(
                            out=v4[:], in0=v4[:],
                            in1=b_row[:].to_broadcast([P, GH]), op=AOp.add)
                    svg = work.tile([P, B_GRP], F32, tag="svg")
                    nc.vector.tensor_reduce(
                        out=svg[:], in_=v4[:].rearrange("p (b d) -> p b d",
                                                        d=H),
                        axis=AxX, op=AOp.add)
                    ng4 = work.tile([P, B_GRP], F32, tag="ng4")
                    nc.scalar.activation(out=ng4[:], in_=svg[:],
                                         func=Act.Copy, scale=-1.0 / H)
                    vmc = work.tile([P, GH], F32, tag="vmc")
                    nc.vector.tensor_tensor(
                        out=vmc[:].rearrange("p (b d) -> p b d", d=H),
                        in0=v4[:].rearrange("p (b d) -> p b d", d=H),
                        in1=ng4[:].to_broadcast([P, B_GRP, H]), op=AOp.add)
                    sq4 = work.tile([P, GH], F32, tag="sq4")
                    nc.scalar.activation(out=sq4[:], in_=vmc[:],
                                         func=Act.Square)
                    vs4 = work.tile([P, B_GRP], F32, tag="vs4")
                    nc.vector.tensor_reduce(
                        out=vs4[:], in_=sq4[:].rearrange("p (b d) -> p b d",
                                                         d=H),
                        axis=AxX, op=AOp.add)
                    vsad = work.tile([P, B_GRP], F32, tag="vsad")
                    nc.vector.tensor_tensor(
                        out=vsad[:], in0=vs4[:],
                        in1=epsdegH_t[:, g * B_GRP:(g + 1) * B_GRP],
                        op=AOp.add)
                    sd4 = work.tile([P, B_GRP], F32, tag="sd4")
                    nc.scalar.activation(out=sd4[:], in_=vsad[:],
                                         func=Act.Sqrt, scale=1.0 / H)
                    rr4 = work.tile([P, B_GRP], F32, tag="rr4")
                    nc.vector.reciprocal(out=rr4[:], in_=sd4[:])
                    h4 = work.tile([P, GH], F16 if fast else F32, tag="h4")
                    nc.vector.tensor_tensor(
                        out=h4[:].rearrange("p (b d) -> p b d", d=H),
                        in0=vmc[:].rearrange("p (b d) -> p b d", d=H),
                        in1=rr4[:].to_broadcast([P, B_GRP, H]), op=AOp.mult)
                    if not fast:
                        lw = rows["ln1wr" if lyr == 1 else "ln2wr"]
                        lbr = rows["ln1br" if lyr == 1 else "ln2br"]
                        nc.vector.tensor_tensor(
                            out=h4[:], in0=h4[:],
                            in1=lw[:].to_broadcast([P, GH]), op=AOp.mult)
                        nc.vector.tensor_tensor(
                            out=h4[:], in0=h4[:],
                            in1=lbr[:].to_broadcast([P, GH]), op=AOp.add)
                    # ELU(h) = min(exp(h) - 1, relu(h))
                    ex4 = work.tile([P, GH], F32, tag="ex4")
                    nc.scalar.activation(out=ex4[:], in_=h4[:], func=Act.Exp)
                    rl4 = work.tile([P, GH], F16, tag="rl4")
                    nc.vector.tensor_scalar_max(out=rl4[:], in0=h4[:],
                                                scalar1=0.0)
                    helu4 = work.tile([P, GH], F16, tag="helu4")
                    nc.vector.scalar_tensor_tensor(
                        out=helu4[:], in0=ex4[:], scalar=1.0, in1=rl4[:],
                        op0=AOp.subtract, op1=AOp.min)

                    if lyr == 1:
                        # ---- proj2 for this group: xw2 = dinv * (h @ W2)
                        pg = psum_t.tile([P, GH], F32, space="PSUM", tag="pg",
                                         name="pg")
                        for bl in range(B_GRP):
                            pst = psum_t.tile([P, P], F16, space="PSUM",
                                              tag="tp", name="pst")
                            nc.tensor.transpose(
                                out=pst[:], in_=helu4[:, bl * H:(bl + 1) * H],
                                identity=ident_h[:])
                            hT = work.tile([H, P], F16, tag="hT", bufs=2,
                                           name="hT")
                            nc.scalar.activation(out=hT[:], in_=pst[:],
                                                 func=Act.Copy)
                            nc.tensor.matmul(out=pg[:, bl * H:(bl + 1) * H],
                                             lhsT=hT[:], rhs=W2_t[:],
                                             start=True, stop=True)
                        # table2 rows are prescaled by dinv[src] (coef
                        # separability), matching the layer-1 host prescale
                        nc.vector.tensor_tensor(
                            out=selfres2[g][:].rearrange("p (b d) -> p b d",
                                                         d=H),
                            in0=pg[:].rearrange("p (b d) -> p b d", d=H),
                            in1=d1t_t[:, g * B_GRP:(g + 1) * B_GRP]
                            .to_broadcast([P, B_GRP, H]),
                            op=AOp.mult)
                        nc.sync.dma_start(
                            out=ag2_in[g * GH:(g + 1) * GH]
                            .rearrange("(b p) d -> p b d", p=P),
                            in_=selfres2[g][:].rearrange("p (b d) -> p b d",
                                                         d=H))
                    else:
                        # ---- bcos residual mix + pooling
                        sq2 = work.tile([P, GH], F32, tag="sq4", name="sq2")
                        nc.scalar.activation(out=sq2[:], in_=helu4[:],
                                             func=Act.Square)
                        qs4 = work.tile([P, B_GRP], F32, tag="qs4")
                        nc.vector.tensor_reduce(
                            out=qs4[:],
                            in_=sq2[:].rearrange("p (b d) -> p b d", d=H),
                            axis=AxX, op=AOp.add)
                        nrm4 = work.tile([P, B_GRP], F32, tag="nrm4")
                        nc.scalar.activation(out=nrm4[:], in_=qs4[:],
                                             func=Act.Sqrt,
                                             bias=bcos_eps_t[:])
                        den4 = work.tile([P, B_GRP], F32, tag="den4")
                        nc.vector.tensor_scalar_add(out=den4[:], in0=nrm4[:],
                                                    scalar1=BCOS_EPS)
                        rcp4 = work.tile([P, B_GRP], F32, tag="rcp4")
                        nc.vector.reciprocal(out=rcp4[:], in_=den4[:])
                        fac4 = work.tile([P, B_GRP], F32, tag="fac4")
                        nc.scalar.activation(out=fac4[:], in_=rcp4[:],
                                             func=Act.Copy,
                                             scale=(1.0 - RR) * TEMP, bias=RR)
                        hb4 = work.tile([P, GH], F16, tag="hb4")
                        nc.vector.tensor_tensor(
                            out=hb4[:].rearrange("p (b d) -> p b d", d=H),
                            in0=helu4[:].rearrange("p (b d) -> p b d", d=H),
                            in1=fac4[:].to_broadcast([P, B_GRP, H]),
                            op=AOp.mult)
                        Sp4 = spool.tile([P, B_GRP * P], F16, tag="Sp4",
                                         bufs=2, name="Sp4")
                        nc.vector.tensor_tensor(
                            out=Sp4[:].rearrange("p (b d) -> p b d", d=P),
                            in0=iotaP16[:].rearrange("p (b d) -> p b d", d=P),
                            in1=lbt_t[:, g * B_GRP:(g + 1) * B_GRP]
                            .to_broadcast([P, B_GRP, P]),
                            op=AOp.is_equal)
                        for bl in range(B_GRP):
                            lb = g * B_GRP + bl
                            nc.tensor.matmul(
                                out=pool_ps[:],
                                lhsT=Sp4[:, bl * P:(bl + 1) * P],
                                rhs=hb4[:, bl * H:(bl + 1) * H],
                                start=(lb == 0),
                                stop=(lb == BLOCKS_PER_CORE - 1))

            with nc.named_scope("layer1"):
                layer(1, idx1_all, t1res, None)

            idx2_all = consts.tile([P, N_GRP * RES * CW], I16, tag="idx",
                                   name="idx2_all")
            nc.sync.dma_start(out=idx2_all[:],
                              in_=idxw2_p[:].rearrange("c p w -> p c w"))

            with nc.named_scope("ag2"):
                nc.gpsimd.collective_compute(
                    "AllGather", AOp.bypass,
                    replica_groups=[list(range(NCORES))],
                    ins=[ag2_in[:].opt()], outs=[tables2[:].opt()],
                )

            pool_ps = psum_t.tile([P, H], F32, space="PSUM", tag="pg",
                                  name="pool_ps")
            with nc.named_scope("layer2"):
                layer(2, idx2_all, t2res, pool_ps)

            # ------------ pooled partial -> transpose -> classifier
            with nc.named_scope("cls"):
                pooled = work.tile([P, H], F32, tag="pooled")
                nc.vector.tensor_copy(out=pooled[:], in_=pool_ps[:])
                psT = psum_t.tile([P, P], F32, space="PSUM", tag="tp",
                                  name="psT")
                nc.tensor.transpose(out=psT[:], in_=pooled[:],
                                    identity=ident_f[:])
                pooledT = work.tile([P, P], F32, tag="pooledT")
                nc.vector.tensor_copy(out=pooledT[:], in_=psT[:])
                cls_ps = psum_t.tile([P, C], F32, space="PSUM", tag="pg",
                                     name="cls_ps")
                nc.tensor.matmul(out=cls_ps[:], lhsT=pooledT[:], rhs=WnT_t[:],
                                 start=True, stop=True)
                outt = work.tile([P, C], F32, tag="outt")
                nc.vector.tensor_copy(out=outt[:], in_=cls_ps[:])
                nc.sync.dma_start(out=out_p[:], in_=outt[:])

    nc.finalize()
    return nc


_CACHE: dict = {}
LAST_RESULTS = None


def _ensure_ntff_hook():
    """Install the antenv.axon_hooks shim so trace=True captures NTFF
    profiles through the axon PJRT .so (the trimmed container lacks the
    module trn_boot expects)."""
    import sys as _sys
    import types

    if "antenv.axon_hooks" not in _sys.modules:
        mod = types.ModuleType("antenv.axon_hooks")
        holder = [None]
        mod.set_axon_ntff_profile_hook = lambda h: holder.__setitem__(0, h)
        mod.get_axon_ntff_profile_hook = lambda: holder[0]
        _sys.modules["antenv.axon_hooks"] = mod
        import antenv

        antenv.axon_hooks = mod
    from antenv.axon_hooks import (get_axon_ntff_profile_hook,
                                   set_axon_ntff_profile_hook)

    if get_axon_ntff_profile_hook() is None:
        from trn_agent_boot.trn_boot import _ntff_profile_via_ctypes

        h = _ntff_profile_via_ctypes("/opt/axon/libaxon_pjrt.so")
        if h is not None:
            set_axon_ntff_profile_hook(h)


def kernel(**inputs) -> np.ndarray:
    np_inputs = {k: np.asarray(v) for k, v in inputs.items()}
    prep = _prep(**np_inputs)
    K = prep["K"]
    tkey = (K, tuple(sorted(prep["trivial"].items())))
    if tkey not in _CACHE:
        _CACHE[tkey] = _build(K, prep["trivial"])
    nc = _CACHE[tkey]

    in_maps = []
    for c in range(NCORES):
        in_maps.append(dict(
            xsT=prep["xsT"][c], W1h=prep["W1h"], W2h=prep["W2h"],
            idxw1=prep["idxw1"][c], idxw2=prep["idxw2"][c],
            ldt=prep["ldt"][c], epsdegH=prep["epsdegH"][c],
            d1t=prep["d1t"][c], lbt=prep["lbt"][c], WnT=prep["WnT"],
            iota16=prep["iota16"], iotaP16=prep["iotaP16"],
            b1r=prep["b1"][None, :], b2r=prep["b2"][None, :],
            ln1wr=prep["ln1_w"][None, :], ln1br=prep["ln1_b"][None, :],
            ln2wr=prep["ln2_w"][None, :], ln2br=prep["ln2_b"][None, :],
        ))
    import os
    trace = bool(os.environ.get("BASS_KERNEL_TRACE"))
    if trace:
        _ensure_ntff_hook()
    res = run_bass_kernel_spmd(nc, in_maps, core_ids=list(range(NCORES)),
                               trace=trace)
    global LAST_RESULTS
    LAST_RESULTS = res
    if trace and res.exec_time_ns is not None:
        print(f"HW exec time: {res.exec_time_ns} ns", flush=True)

    # host unshard: scatter-add partial logits by per-core graph base,
    # divide by graph node counts, add classifier bias
    out = np.zeros((G, C), np.float64)
    for c in range(NCORES):
        part = res.results[c]["out_part"].astype(np.float64)
        gb = int(prep["g_base"][c])
        hi = min(G, gb + P)
        out[gb:hi] += part[: hi - gb]
    out = out / prep["cnt"][:, None] + prep["cls_b"][None, :]
    return out.astype(np.float32)


# revision 40
# speedup vs baseline: 1.3988x; 1.0248x over previous
"""Trainium2 Bass kernel for a 2-layer BCos-GCN (nn_BCosGCN_28346784153649).

Strategy (8 NeuronCores, SPMD):
  - Nodes (and their incident edges, grouped by destination block) are
    sharded across the 8 cores; the 128x128 weights are replicated.
  - Tables are ONE flat fp16 [102400, 128] tensor per layer; the 4 "residue
    banks" (slot % 4) are stride-4 row views of it, so int16 gather indices
    stay < 25600 while projection writes / self-row reads are single big
    DMAs.
  - Layer 1: the projection (dinv*x) @ W1 is REPLICATED on every core from a
    host-staged fp16 x^T (rotated per core so each core's own nodes occupy
    flat-table rows [0, 12800)) -- no collective needed.
  - Aggregation per layer: dma_gather source rows by residue bank and
    accumulate per 128-destination-node block via one-hot matmuls (PSUM
    accumulation).  The one-hot S matrices are built ON-CHIP with one
    is_equal per 4-block group against an iota constant.
  - LayerNorm scale-invariance: b1/b2 are zero, so the dinv[dst] scaling
    before LN is folded away exactly via a per-node H*eps*deg bias added to
    the variance sum.
  - Layer 2 projection runs inside the layer-1 loop per 4-block group; ONE
    AllGather (Shared output) assembles the rank-major layer-2 table; the
    layer-2 self rows stay resident in SBUF.
  - Global mean-pool via one-hot matmul accumulation + weight-normalized
    classifier; tiny [128, 10] per-core partials are combined on the host.
"""

import sys

sys.path.insert(0, "/opt/trn_rl_repo")

import numpy as np

from concourse import bacc, tile, mybir
from concourse.bass_utils import run_bass_kernel_spmd
from concourse.masks import make_identity

# ---------------------------------------------------------------- constants
N, E, F, H, C, G = 100000, 1600000, 128, 128, 10, 512
LN_EPS = 1e-5
BCOS_EPS = 1e-6
TEMP = 1.5
RR = 0.6  # residual ratio; bcos exponent B == 1.0 -> bcos(h) = TEMP*h/(nrm+eps)

NCORES = 8
P = 128
REAL_PER_CORE = N // NCORES            # 12500
NODES_PER_CORE = 12800                 # padded: 100 blocks of 128
BLOCKS_PER_CORE = NODES_PER_CORE // P  # 100
NPAD = NODES_PER_CORE * NCORES         # 102400
NBLK = NPAD // P                       # 800
RES = 4                                # residue banks (slot % 4)
B_GRP = 4                              # dst blocks per gather call / group
N_GRP = BLOCKS_PER_CORE // B_GRP       # 25 groups per core
ROWS_RES = NPAD // RES                 # 25600 rows per residue view
PJ = 4                                 # blocks per proj1 iteration

F16 = mybir.dt.float16
F32 = mybir.dt.float32
I16 = mybir.dt.int16
I32 = mybir.dt.int32
AOp = mybir.AluOpType
Act = mybir.ActivationFunctionType
AxX = mybir.AxisListType.X


# ---------------------------------------------------------------- host prep
def _lpt_blocks(indeg_core: np.ndarray) -> list[list[int]]:
    """Pack the core's real nodes into 100 blocks of <=128, balancing the
    in-degree sum per block (greedy LPT)."""
    import heapq

    order = np.argsort(-indeg_core, kind="stable")
    heap = [(0, 0, b) for b in range(BLOCKS_PER_CORE)]
    heapq.heapify(heap)
    blocks: list[list[int]] = [[] for _ in range(BLOCKS_PER_CORE)]
    for v in order:
        while True:
            load, cnt, b = heapq.heappop(heap)
            if cnt < P:
                break
        blocks[b].append(int(v))
        heapq.heappush(heap, (load + int(indeg_core[v]), cnt + 1, b))
    return blocks


def _color_banks(ownblk, src, dstblk, rounds=24, seed=0):
    """Greedy residue-bank coloring balancing (dst-block, color) edge cells
    at <=512 (-> K=4), subject to <=32 nodes per (own-block, color)."""
    SLOT_CAP = P // RES
    Nn = ownblk.shape[0]
    rng = np.random.default_rng(seed)
    eorder = np.argsort(src, kind="stable")
    e_dstblk = dstblk[eorder]
    esrc = src[eorder]
    degn = np.bincount(src, minlength=Nn)
    estart = np.concatenate([[0], np.cumsum(degn)])
    cellcnt = np.zeros((NBLK, RES), np.int64)
    slotcnt = np.zeros((NBLK, RES), np.int32)
    color = np.full(Nn, -1, np.int32)
    order = np.argsort(-degn, kind="stable")
    target = max(1.0, dstblk.shape[0] / (NBLK * RES))
    cap = int(np.ceil(target / P) * P)
    for bt in np.array_split(order, rounds):
        nb = bt.shape[0]
        reps = degn[bt]
        node_rep = np.repeat(np.arange(nb), reps)
        eidx = (np.concatenate([np.arange(estart[v], estart[v + 1]) for v in bt])
                if nb else np.empty(0, np.int64))
        score = np.zeros((nb, RES), np.float64)
        if eidx.size:
            np.add.at(score, node_rep, cellcnt[e_dstblk[eidx]])
        own = ownblk[bt]
        score += np.where(slotcnt[own] >= SLOT_CAP, 1e12, 0.0)
        if eidx.size:
            np.add.at(score, node_rep,
                      np.where(cellcnt[e_dstblk[eidx]] >= cap - 1, 1e6, 0.0))
        score += rng.random((nb, RES))
        ch = np.argmin(score, axis=1).astype(np.int32)
        for i in range(nb):
            o, c = own[i], ch[i]
            if slotcnt[o, c] >= SLOT_CAP:
                c = int(np.argmin(slotcnt[o] + np.where(
                    slotcnt[o] >= SLOT_CAP, 10**9, 0)))
                ch[i] = c
            slotcnt[o, c] += 1
        color[bt] = ch
        if eidx.size:
            np.add.at(cellcnt, (e_dstblk[eidx], ch[node_rep]), 1)
    # exact repair: move nodes out of over-cap cells
    border = np.argsort(e_dstblk, kind="stable")
    bcnt = np.bincount(e_dstblk, minlength=NBLK)
    bstart = np.concatenate([[0], np.cumsum(bcnt)])
    for _ in range(40):
        over = np.argwhere(cellcnt > cap)
        if over.size == 0:
            break
        for bb, cc in over:
            while cellcnt[bb, cc] > cap:
                cands = np.unique(esrc[border[bstart[bb]:bstart[bb + 1]]])
                cands = cands[color[cands] == cc]
                moved = False
                contrib = np.array([
                    np.count_nonzero(e_dstblk[estart[v]:estart[v + 1]] == bb)
                    for v in cands])
                for v in cands[np.argsort(contrib)]:
                    o = ownblk[v]
                    blks = e_dstblk[estart[v]:estart[v + 1]]
                    for c2 in np.argsort(cellcnt[bb]):
                        if c2 == cc or slotcnt[o, c2] >= SLOT_CAP:
                            continue
                        add = np.bincount(blks, minlength=NBLK)
                        touched = np.nonzero(add)[0]
                        if (cellcnt[touched, c2] + add[touched] <= cap).all():
                            cellcnt[touched, cc] -= add[touched]
                            cellcnt[touched, c2] += add[touched]
                            slotcnt[o, cc] -= 1
                            slotcnt[o, c2] += 1
                            color[v] = c2
                            moved = True
                            break
                    if moved:
                        break
                if not moved:
                    break
    return color


def _prep(x, src, dst, batch, W1, b1, ln1_w, ln1_b, W2, b2, ln2_w, ln2_b,
          cls_v, cls_g, cls_b, seed=0):
    indeg = np.bincount(dst, minlength=N)
    deg = indeg.astype(np.float32) + 1.0
    dinv = (1.0 / np.sqrt(deg)).astype(np.float32)

    # ---- node -> (core, block); LPT balance in-degree per block
    ownblk = np.zeros(N, np.int64)
    core_blocks = []
    g_base = np.zeros(NCORES, np.int64)
    for c in range(NCORES):
        lo, hi = c * REAL_PER_CORE, (c + 1) * REAL_PER_CORE
        g_base[c] = int(batch[lo])
        span = int(batch[hi - 1]) - g_base[c]
        assert span < P, f"core {c} spans {span + 1} graphs > 128"
        blocks = _lpt_blocks(indeg[lo:hi])
        core_blocks.append(blocks)
        for b in range(BLOCKS_PER_CORE):
            for v_local in blocks[b]:
                ownblk[lo + v_local] = c * BLOCKS_PER_CORE + b

    # ---- residue-bank coloring (cells <= 512 -> K=4); slot assignment
    # color r occupies slots {r, r+4, ...}: residue class = slot % 4, so the
    # 4 residue banks are stride-4 row views of ONE flat table.
    s64 = src.astype(np.int64)
    d64 = dst.astype(np.int64)
    color = _color_banks(ownblk, s64, ownblk[d64])
    pos = np.full(N, -1, np.int64)
    for c in range(NCORES):
        lo = c * REAL_PER_CORE
        for b in range(BLOCKS_PER_CORE):
            blk = core_blocks[c][b]
            base = c * NODES_PER_CORE + b * P
            nxt = [0, 0, 0, 0]
            for v_local in blk:
                cc = int(color[lo + v_local])
                sl = cc + RES * nxt[cc]
                nxt[cc] += 1
                pos[lo + v_local] = base + sl
    assert (pos >= 0).all()

    # ---- per-position node data (pad positions keep zeros / neutral values)
    node_at = np.full(NPAD, -1, np.int64)
    node_at[pos] = np.arange(N)
    ok = node_at >= 0
    sel = node_at[ok]

    # x^T, prescaled by dinv; per-core rotated copies (own nodes first)
    xpos = np.zeros((NPAD, F), np.float32)
    xpos[ok] = x[sel] * dinv[sel][:, None]
    xsTg = np.ascontiguousarray(xpos.T.astype(np.float16))       # [F, NPAD]
    xsT = [np.ascontiguousarray(np.roll(xsTg, -NODES_PER_CORE * c, axis=1))
           for c in range(NCORES)]

    degpos = np.ones(NPAD, np.float32)
    degpos[ok] = deg[sel]
    batpos = np.zeros(NPAD, np.float32)
    batpos[ok] = batch[sel].astype(np.float32)

    trivial = dict(
        b1=not np.any(b1), b2=not np.any(b2),
        ln1=bool(np.all(ln1_w == 1.0) and not np.any(ln1_b)),
        ln2=bool(np.all(ln2_w == 1.0) and not np.any(ln2_b)),
    )
    fast = all(trivial.values())

    d1t = np.zeros((NCORES, P, BLOCKS_PER_CORE), np.float32)
    epsdegH = np.zeros((NCORES, P, BLOCKS_PER_CORE), np.float32)
    lbt = np.zeros((NCORES, P, BLOCKS_PER_CORE), np.float16)
    for c in range(NCORES):
        sl = slice(c * NODES_PER_CORE, (c + 1) * NODES_PER_CORE)
        d1 = 1.0 / np.sqrt(degpos[sl])
        d1t[c] = d1.reshape(BLOCKS_PER_CORE, P).T
        # variance correction: sqrt((vs + H*eps*deg)/H) == sqrt(vs/H + eps*deg)
        ed = (H * LN_EPS * degpos[sl]) if fast else np.full(
            NODES_PER_CORE, H * LN_EPS, np.float32)
        epsdegH[c] = ed.reshape(BLOCKS_PER_CORE, P).T
        lb = (batpos[sl] - g_base[c]).astype(np.float16)
        lbt[c] = lb.reshape(BLOCKS_PER_CORE, P).T

    # ---- edges -> cells (dst block x src residue class), padded to K*128
    pe_src = pos[s64]
    pe_dst = pos[d64]
    blk = pe_dst >> 7
    slot_s = pe_src & 127
    res = slot_s % RES
    idx_g = ((pe_src >> 7) * 32 + (slot_s >> 2)).astype(np.int64)  # view row
    ld = (pe_dst & 127).astype(np.float16)
    cell = blk * RES + res
    counts = np.bincount(cell, minlength=NBLK * RES)
    K = int(np.ceil(counts.max() / P))
    CELL = K * P

    order = np.argsort(cell, kind="stable")
    starts = np.cumsum(counts) - counts
    within = np.arange(E) - np.repeat(starts, counts)
    flat = cell[order] * CELL + within
    # pad slots gather a zero table row (a pad node) of the right residue
    apos = np.arange(NPAD)
    padrow = np.zeros(RES, np.int64)
    for rr_ in range(RES):
        cand = np.nonzero((((apos & 127) % RES) == rr_) & (node_at < 0))[0]
        pp = int(cand[0])
        padrow[rr_] = (pp >> 7) * 32 + ((pp & 127) >> 2)
    idxA = np.tile(np.repeat(padrow, CELL), NBLK)
    ldA = np.full(NBLK * RES * CELL, -1.0, np.float16)
    idxA[flat] = idx_g[order]
    ldA[flat] = ld[order]
    idxA = idxA.reshape(NBLK, RES, CELL)
    ldA = ldA.reshape(NBLK, RES, K, P)

    call_len = B_GRP * CELL
    idxw1 = np.zeros((NCORES, N_GRP * RES, P, call_len // 16), np.int16)
    idxw2 = np.zeros((NCORES, N_GRP * RES, P, call_len // 16), np.int16)
    ldt = np.zeros((NCORES, P, BLOCKS_PER_CORE * RES * K), np.float16)
    for c in range(NCORES):
        b0 = c * BLOCKS_PER_CORE
        idxc = (idxA - 3200 * c) % ROWS_RES  # rotated view rows
        for g in range(N_GRP):
            for rr in range(RES):
                l1 = idxc[b0 + g * B_GRP: b0 + (g + 1) * B_GRP, rr, :].reshape(-1)
                l2 = idxA[b0 + g * B_GRP: b0 + (g + 1) * B_GRP, rr, :].reshape(-1)
                w1 = l1.astype(np.int16).reshape(-1, 16).T
                w2 = l2.astype(np.int16).reshape(-1, 16).T
                idxw1[c, g * RES + rr] = np.tile(w1, (8, 1))
                idxw2[c, g * RES + rr] = np.tile(w2, (8, 1))
        # ldt[c][p, b*RES*K + rr*K + k] = ldA[b0+b, rr, k, p]
        ldt[c] = ldA[b0:b0 + BLOCKS_PER_CORE].reshape(
            BLOCKS_PER_CORE * RES * K, P).T

    # ---- classifier / epilogue host data
    WnT = (cls_g[:, None] * cls_v
           / np.linalg.norm(cls_v, axis=1, keepdims=True)).T.astype(np.float32)
    cnt = np.maximum(np.bincount(batch, minlength=G).astype(np.float32), 1.0)

    iota16 = np.tile(np.arange(P, dtype=np.float16),
                     (P, B_GRP * RES * K)).reshape(P, -1)
    iotaP16 = np.tile(np.arange(P, dtype=np.float16),
                      (P, B_GRP)).reshape(P, -1)

    return dict(
        K=K, xsT=xsT, d1t=d1t, epsdegH=epsdegH, lbt=lbt,
        idxw1=idxw1, idxw2=idxw2, ldt=ldt, iota16=iota16, iotaP16=iotaP16,
        WnT=WnT, cnt=cnt, g_base=g_base, trivial=trivial,
        W1h=W1.astype(np.float16), W2h=W2.astype(np.float16),
        b1=b1.astype(np.float32), b2=b2.astype(np.float32),
        ln1_w=ln1_w.astype(np.float32), ln1_b=ln1_b.astype(np.float32),
        ln2_w=ln2_w.astype(np.float32), ln2_b=ln2_b.astype(np.float32),
        cls_b=cls_b.astype(np.float32),
    )


# ---------------------------------------------------------------- program
def _build(K: int, trivial: dict):
    CELL = K * P
    call_len = B_GRP * CELL
    CW = call_len // 16
    RK = RES * K
    GH = B_GRP * H
    GRK = B_GRP * RK
    fast = all(trivial.values())

    nc = bacc.Bacc(None, target_bir_lowering=False, debug=False,
                   num_devices=NCORES, num_swdge_queues=4,
                   dynamic_dma_scratch_size=24576)

    xsT_p = nc.declare_dram_parameter("xsT", [F, NPAD], F16, isOutput=False)
    W1_p = nc.declare_dram_parameter("W1h", [F, H], F16, isOutput=False)
    W2_p = nc.declare_dram_parameter("W2h", [H, H], F16, isOutput=False)
    idxw1_p = nc.declare_dram_parameter(
        "idxw1", [N_GRP * RES, P, CW], I16, isOutput=False)
    idxw2_p = nc.declare_dram_parameter(
        "idxw2", [N_GRP * RES, P, CW], I16, isOutput=False)
    ldt_p = nc.declare_dram_parameter(
        "ldt", [P, BLOCKS_PER_CORE * RK], F16, isOutput=False)
    epsdegH_p = nc.declare_dram_parameter(
        "epsdegH", [P, BLOCKS_PER_CORE], F32, isOutput=False)
    d1t_p = nc.declare_dram_parameter("d1t", [P, BLOCKS_PER_CORE], F32, isOutput=False)
    lbt_p = nc.declare_dram_parameter("lbt", [P, BLOCKS_PER_CORE], F16, isOutput=False)
    WnT_p = nc.declare_dram_parameter("WnT", [H, C], F32, isOutput=False)
    iota16_p = nc.declare_dram_parameter(
        "iota16", [P, B_GRP * RES * K * P], F16, isOutput=False)
    iotaP16_p = nc.declare_dram_parameter(
        "iotaP16", [P, B_GRP * P], F16, isOutput=False)
    rows_p = {}
    for nm in ["b1r", "b2r", "ln1wr", "ln1br", "ln2wr", "ln2br"]:
        rows_p[nm] = nc.declare_dram_parameter(nm, [1, H], F32, isOutput=False)
    out_p = nc.declare_dram_parameter("out_part", [P, C], F32, isOutput=True)

    with tile.TileContext(nc, num_cores=NCORES) as tc:
        with (
            tc.tile_pool(name="consts", bufs=1) as consts,
            tc.tile_pool(name="resident", bufs=1) as resident,
            tc.tile_pool(name="work", bufs=3) as work,
            tc.tile_pool(name="gat", bufs=2) as gatp,
            tc.tile_pool(name="spool", bufs=2) as spool,
            tc.tile_pool(name="psum_agg", bufs=3, space="PSUM") as psum_agg,
            tc.tile_pool(name="psum_pj", bufs=3, space="PSUM") as psum_pj,
            tc.tile_pool(name="psum_t", bufs=1, space="PSUM") as psum_t,
            tc.tile_pool(name="dram", bufs=1, space="DRAM") as dram,
        ):
            tables1 = dram.tile([NPAD, H], F16, tag="t1", name="t1")
            tables2 = dram.tile([NPAD, H], F16, tag="t2", name="t2",
                                addr_space="Shared")
            ag2_in = dram.tile([NODES_PER_CORE, H], F16, tag="ag2i",
                               name="ag2i")

            # stride-4 residue-bank row views for the gathers
            def res_views(tab):
                quad = tab[:].rearrange("(n four) d -> n (four d)", four=RES)
                return [quad[:, r * H:(r + 1) * H] for r in range(RES)]

            t1res = res_views(tables1)
            t2res = res_views(tables2)

            # ---------------- constants
            W1_t = consts.tile([F, H], F16)
            nc.sync.dma_start(out=W1_t[:], in_=W1_p[:])
            W2_t = consts.tile([H, H], F16)
            nc.sync.dma_start(out=W2_t[:], in_=W2_p[:])
            epsdegH_t = consts.tile([P, BLOCKS_PER_CORE], F32)
            nc.sync.dma_start(out=epsdegH_t[:], in_=epsdegH_p[:])
            lbt_t = consts.tile([P, BLOCKS_PER_CORE], F16)
            nc.sync.dma_start(out=lbt_t[:], in_=lbt_p[:])
            WnT_t = consts.tile([H, C], F32)
            nc.sync.dma_start(out=WnT_t[:], in_=WnT_p[:])
            ldt_t = consts.tile([P, BLOCKS_PER_CORE * RK], F16)
            nc.sync.dma_start(out=ldt_t[:], in_=ldt_p[:])
            d1t_t = consts.tile([P, BLOCKS_PER_CORE], F32)
            nc.sync.dma_start(out=d1t_t[:], in_=d1t_p[:])
            rows = {}
            if not fast:
                for nm, pp in rows_p.items():
                    t = consts.tile([1, H], F32, tag=f"row_{nm}",
                                    name=f"row_{nm}")
                    nc.sync.dma_start(out=t[:], in_=pp[:])
                    rows[nm] = t

            # both gather index sets share one SBUF slot; the idx2 load waits
            # (via WAR) until the last layer-1 gather has issued
            idx1_all = consts.tile([P, N_GRP * RES * CW], I16, tag="idx",
                                   name="idx1_all")
            nc.sync.dma_start(out=idx1_all[:],
                              in_=idxw1_p[:].rearrange("c p w -> p c w"))

            ident_h = consts.tile([P, P], F16)
            make_identity(nc, ident_h[:])
            ident_f = consts.tile([P, P], F32)
            make_identity(nc, ident_f[:])

            # iota constants: value = position within each 128 chunk
            iota16 = consts.tile([P, GRK * P], F16)
            nc.sync.dma_start(out=iota16[:], in_=iota16_p[:])
            iotaP16 = consts.tile([P, B_GRP * P], F16)
            nc.sync.dma_start(out=iotaP16[:], in_=iotaP16_p[:])

            bcos_eps_t = consts.tile([P, 1], F32)
            nc.vector.memset(bcos_eps_t[:], BCOS_EPS)
            zeros_c = consts.tile([P, 1], F16)
            nc.vector.memset(zeros_c[:], 0.0)

            # layer-2 self rows (own xw2), kept resident between the loops
            selfres2 = [resident.tile([P, GH], F16, tag=f"s2g{g}",
                                      name=f"s2g{g}")
                        for g in range(N_GRP)]

            # ---------------- proj1: replicated (dinv*x) @ W1 -> tables1
            with nc.named_scope("proj1"):
                for i in range(NBLK // PJ):
                    xt4 = work.tile([F, PJ * P], F16, tag="xt4")
                    nc.sync.dma_start(
                        out=xt4[:], in_=xsT_p[:, i * PJ * P:(i + 1) * PJ * P])
                    pp = psum_pj.tile([P, PJ * H], F32, space="PSUM", tag="pj")
                    for bl in range(PJ):
                        nc.tensor.matmul(out=pp[:, bl * H:(bl + 1) * H],
                                         lhsT=xt4[:, bl * P:(bl + 1) * P],
                                         rhs=W1_t[:], start=True, stop=True)
                    xw4 = work.tile([P, PJ * H], F16, tag="xw4")
                    if i % 2 == 0:
                        nc.scalar.activation(out=xw4[:], in_=pp[:],
                                             func=Act.Copy)
                    else:
                        nc.vector.tensor_copy(out=xw4[:], in_=pp[:])
                    nc.sync.dma_start(
                        out=tables1[i * PJ * P:(i + 1) * PJ * P]
                        .rearrange("(b p) d -> p b d", p=P),
                        in_=xw4[:].rearrange("p (b d) -> p b d", d=H))

            # ---------------- one GCN layer: gather + agg + LN + ELU
            def layer(lyr, idx_all, tabs, pool_ps):
                for g in range(N_GRP):
                    gtiles = []
                    for rr in range(RES):
                        ci = g * RES + rr
                        gt = gatp.tile([P, B_GRP * K, H], F16, tag=f"gat{rr}",
                                       bufs=2, name=f"gat{rr}")
                        nc.gpsimd.dma_gather(
                            out_ap=gt[:], in_ap=tabs[rr],
                            idxs_ap=idx_all[:, ci * CW:(ci + 1) * CW],
                            num_idxs=call_len,
                            num_idxs_reg=call_len, elem_size=H,
                            elem_step=RES * H, single_packet=False,
                            queue_num=rr,
                        )
                        gtiles.append(gt)

                    # self rows: layer1 reads own table rows; layer2 has them
                    if lyr == 1:
                        selfb4 = work.tile([P, B_GRP, H], F16, tag="selfb4")
                        nc.sync.dma_start(
                            out=selfb4[:],
                            in_=tables1[g * GH:(g + 1) * GH]
                            .rearrange("(b p) d -> p b d", p=P))
                        selfv = selfb4[:].rearrange("p b d -> p (b d)")
                    else:
                        selfv = selfres2[g][:]

                    # one-hot S for the whole group with a single is_equal
                    Sbig = spool.tile([P, GRK * P], F16, tag="Sbig",
                                      bufs=2, name="Sbig")
                    nc.vector.tensor_tensor(
                        out=Sbig[:].rearrange("p (j m) -> p j m", m=P),
                        in0=iota16[:].rearrange("p (j m) -> p j m", m=P),
                        in1=ldt_t[:, g * GRK:(g + 1) * GRK]
                        .to_broadcast([P, GRK, P]),
                        op=AOp.is_equal)

                    # aggregation matmuls: one PSUM bank per group.  Each
                    # block's accumulation group stays contiguous (start/stop
                    # semantics require it), but blocks rotate their starting
                    # residue so not every block stalls on the same gather.
                    ps4 = psum_agg.tile([P, GH], F32, space="PSUM",
                                        tag="agg4", name="ps4")
                    for bl in range(B_GRP):
                        for i in range(RES):
                            rr = (bl + i) % RES
                            for k in range(K):
                                j2 = rr * K + k
                                nc.tensor.matmul(
                                    out=ps4[:, bl * H:(bl + 1) * H],
                                    lhsT=Sbig[:, (bl * RK + j2) * P:
                                              (bl * RK + j2 + 1) * P],
                                    rhs=gtiles[rr][:, bl * K + k, :],
                                    start=(i == 0 and k == 0),
                                    stop=(i == RES - 1 and k == K - 1),
                                )

                    # ---- epilogue over the 4 blocks [P, 4H]
                    v4 = work.tile([P, GH], F32, tag="v4")
                    nc.vector.tensor_tensor(out=v4[:], in0=ps4[:], in1=selfv,
                                            op=AOp.add)
                    if not fast:
                        b_row = rows["b1r" if lyr == 1 else "b2r"]
                        nc.vector.tensor_tensor(
                            out=v4[:].rearrange("p (b d) -> p b d", d=H),
                            in0=v4[:].rearrange("p (b d) -> p b d", d=H),
                            in1=d1t_t[:, g * B_GRP:(g + 1) * B_GRP]
                            .to_broadcast([P, B_GRP, H]), op=AOp.mult)
                        nc.vector.tensor_tensor(
                            out=v4[:], in0=v4[:],
                            in1=b_row[:].to_broadcast([P, GH]), op=AOp.add)
                    svg = work.tile([P, B_GRP], F32, tag="svg")
                    nc.vector.tensor_reduce(
                        out=svg[:], in_=v4[:].rearrange("p (b d) -> p b d",
                                                        d=H),
                        axis=AxX, op=AOp.add)
                    ng4 = work.tile([P, B_GRP], F32, tag="ng4")
                    nc.scalar.activation(out=ng4[:], in_=svg[:],
                                         func=Act.Copy, scale=-1.0 / H)
                    vmc = work.tile([P, GH], F32, tag="vmc")
                    nc.vector.tensor_tensor(
                        out=vmc[:].rearrange("p (b d) -> p b d", d=H),
                        in0=v4[:].rearrange("p (b d) -> p b d", d=H),
                        in1=ng4[:].to_broadcast([P, B_GRP, H]), op=AOp.add)
                    sq4 = work.tile([P, GH], F32, tag="sq4")
                    nc.scalar.activation(out=sq4[:], in_=vmc[:],
                                         func=Act.Square)
                    vs4 = work.tile([P, B_GRP], F32, tag="vs4")
                    nc.vector.tensor_reduce(
                        out=vs4[:], in_=sq4[:].rearrange("p (b d) -> p b d",
                                                         d=H),
                        axis=AxX, op=AOp.add)
                    vsad = work.tile([P, B_GRP], F32, tag="vsad")
                    nc.vector.tensor_tensor(
                        out=vsad[:], in0=vs4[:],
                        in1=epsdegH_t[:, g * B_GRP:(g + 1) * B_GRP],
                        op=AOp.add)
                    sd4 = work.tile([P, B_GRP], F32, tag="sd4")
                    nc.scalar.activation(out=sd4[:], in_=vsad[:],
                                         func=Act.Sqrt, scale=1.0 / H)
                    rr4 = work.tile([P, B_GRP], F32, tag="rr4")
                    nc.vector.reciprocal(out=rr4[:], in_=sd4[:])
                    h4 = work.tile([P, GH], F16 if fast else F32, tag="h4")
                    nc.vector.tensor_tensor(
                        out=h4[:].rearrange("p (b d) -> p b d", d=H),
                        in0=vmc[:].rearrange("p (b d) -> p b d", d=H),
                        in1=rr4[:].to_broadcast([P, B_GRP, H]), op=AOp.mult)
                    if not fast:
                        lw = rows["ln1wr" if lyr == 1 else "ln2wr"]
                        lbr = rows["ln1br" if lyr == 1 else "ln2br"]
                        nc.vector.tensor_tensor(
                            out=h4[:], in0=h4[:],
                            in1=lw[:].to_broadcast([P, GH]), op=AOp.mult)
                        nc.vector.tensor_tensor(
                            out=h4[:], in0=h4[:],
                            in1=lbr[:].to_broadcast([P, GH]), op=AOp.add)
                    # ELU(h) = min(exp(h) - 1, relu(h))
                    ex4 = work.tile([P, GH], F32, tag="ex4")
                    nc.scalar.activation(out=ex4[:], in_=h4[:], func=Act.Exp)
                    rl4 = work.tile([P, GH], F16, tag="rl4")
                    nc.vector.tensor_tensor(
                        out=rl4[:], in0=h4[:],
                        in1=zeros_c[:].to_broadcast([P, GH]), op=AOp.max)
                    helu4 = work.tile([P, GH], F16, tag="helu4")
                    nc.vector.scalar_tensor_tensor(
                        out=helu4[:], in0=ex4[:], scalar=1.0, in1=rl4[:],
                        op0=AOp.subtract, op1=AOp.min)

                    if lyr == 1:
                        # ---- proj2 for this group: xw2 = dinv * (h @ W2)
                        pg = psum_t.tile([P, GH], F32, space="PSUM", tag="pg",
                                         name="pg")
                        for bl in range(B_GRP):
                            pst = psum_t.tile([P, P], F16, space="PSUM",
                                              tag="tp", name="pst")
                            nc.tensor.transpose(
                                out=pst[:], in_=helu4[:, bl * H:(bl + 1) * H],
                                identity=ident_h[:])
                            hT = work.tile([H, P], F16, tag="hT", bufs=2,
                                           name="hT")
                            nc.scalar.activation(out=hT[:], in_=pst[:],
                                                 func=Act.Copy)
                            nc.tensor.matmul(out=pg[:, bl * H:(bl + 1) * H],
                                             lhsT=hT[:], rhs=W2_t[:],
                                             start=True, stop=True)
                        # table2 rows are prescaled by dinv[src] (coef
                        # separability), matching the layer-1 host prescale
                        nc.vector.tensor_tensor(
                            out=selfres2[g][:].rearrange("p (b d) -> p b d",
                                                         d=H),
                            in0=pg[:].rearrange("p (b d) -> p b d", d=H),
                            in1=d1t_t[:, g * B_GRP:(g + 1) * B_GRP]
                            .to_broadcast([P, B_GRP, H]),
                            op=AOp.mult)
                        nc.sync.dma_start(
                            out=ag2_in[g * GH:(g + 1) * GH]
                            .rearrange("(b p) d -> p b d", p=P),
                            in_=selfres2[g][:].rearrange("p (b d) -> p b d",
                                                         d=H))
                    else:
                        # ---- bcos residual mix + pooling
                        sq2 = work.tile([P, GH], F32, tag="sq4", name="sq2")
                        nc.scalar.activation(out=sq2[:], in_=helu4[:],
                                             func=Act.Square)
                        qs4 = work.tile([P, B_GRP], F32, tag="qs4")
                        nc.vector.tensor_reduce(
                            out=qs4[:],
                            in_=sq2[:].rearrange("p (b d) -> p b d", d=H),
                            axis=AxX, op=AOp.add)
                        # (nrm + eps) ~= nrm: relative error ~eps/nrm ~ 1e-7
                        nrm4 = work.tile([P, B_GRP], F32, tag="nrm4")
                        nc.scalar.activation(out=nrm4[:], in_=qs4[:],
                                             func=Act.Sqrt,
                                             bias=bcos_eps_t[:])
                        rcp4 = work.tile([P, B_GRP], F32, tag="rcp4")
                        nc.vector.reciprocal(out=rcp4[:], in_=nrm4[:])
                        fac4 = work.tile([P, B_GRP], F32, tag="fac4")
                        nc.scalar.activation(out=fac4[:], in_=rcp4[:],
                                             func=Act.Copy,
                                             scale=(1.0 - RR) * TEMP, bias=RR)
                        hb4 = work.tile([P, GH], F16, tag="hb4")
                        nc.vector.tensor_tensor(
                            out=hb4[:].rearrange("p (b d) -> p b d", d=H),
                            in0=helu4[:].rearrange("p (b d) -> p b d", d=H),
                            in1=fac4[:].to_broadcast([P, B_GRP, H]),
                            op=AOp.mult)
                        Sp4 = spool.tile([P, B_GRP * P], F16, tag="Sp4",
                                         bufs=2, name="Sp4")
                        nc.vector.tensor_tensor(
                            out=Sp4[:].rearrange("p (b d) -> p b d", d=P),
                            in0=iotaP16[:].rearrange("p (b d) -> p b d", d=P),
                            in1=lbt_t[:, g * B_GRP:(g + 1) * B_GRP]
                            .to_broadcast([P, B_GRP, P]),
                            op=AOp.is_equal)
                        for bl in range(B_GRP):
                            lb = g * B_GRP + bl
                            nc.tensor.matmul(
                                out=pool_ps[:],
                                lhsT=Sp4[:, bl * P:(bl + 1) * P],
                                rhs=hb4[:, bl * H:(bl + 1) * H],
                                start=(lb == 0),
                                stop=(lb == BLOCKS_PER_CORE - 1))

            with nc.named_scope("layer1"):
                layer(1, idx1_all, t1res, None)

            idx2_all = consts.tile([P, N_GRP * RES * CW], I16, tag="idx",
                                   name="idx2_all")
            nc.sync.dma_start(out=idx2_all[:],
                              in_=idxw2_p[:].rearrange("c p w -> p c w"))

            with nc.named_scope("ag2"):
                nc.gpsimd.collective_compute(
                    "AllGather", AOp.bypass,
                    replica_groups=[list(range(NCORES))],
                    ins=[ag2_in[:].opt()], outs=[tables2[:].opt()],
                )

            pool_ps = psum_t.tile([P, H], F32, space="PSUM", tag="pg",
                                  name="pool_ps")
            with nc.named_scope("layer2"):
                layer(2, idx2_all, t2res, pool_ps)

            # ------------ pooled partial -> transpose -> classifier
            with nc.named_scope("cls"):
                pooled = work.tile([P, H], F32, tag="pooled")
                nc.vector.tensor_copy(out=pooled[:], in_=pool_ps[:])
                psT = psum_t.tile([P, P], F32, space="PSUM", tag="tp",
                                  name="psT")
                nc.tensor.transpose(out=psT[:], in_=pooled[:],
                                    identity=ident_f[:])
                pooledT = work.tile([P, P], F32, tag="pooledT")
                nc.vector.tensor_copy(out=pooledT[:], in_=psT[:])
                cls_ps = psum_t.tile([P, C], F32, space="PSUM", tag="pg",
                                     name="cls_ps")
                nc.tensor.matmul(out=cls_ps[:], lhsT=pooledT[:], rhs=WnT_t[:],
                                 start=True, stop=True)
                outt = work.tile([P, C], F32, tag="outt")
                nc.vector.tensor_copy(out=outt[:], in_=cls_ps[:])
                nc.sync.dma_start(out=out_p[:], in_=outt[:])

    nc.finalize()
    return nc


_CACHE: dict = {}
LAST_RESULTS = None


def _ensure_ntff_hook():
    """Install the antenv.axon_hooks shim so trace=True captures NTFF
    profiles through the axon PJRT .so (the trimmed container lacks the
    module trn_boot expects)."""
    import sys as _sys
    import types

    if "antenv.axon_hooks" not in _sys.modules:
        mod = types.ModuleType("antenv.axon_hooks")
        holder = [None]
        mod.set_axon_ntff_profile_hook = lambda h: holder.__setitem__(0, h)
        mod.get_axon_ntff_profile_hook = lambda: holder[0]
        _sys.modules["antenv.axon_hooks"] = mod
        import antenv

        antenv.axon_hooks = mod
    from antenv.axon_hooks import (get_axon_ntff_profile_hook,
                                   set_axon_ntff_profile_hook)

    if get_axon_ntff_profile_hook() is None:
        from trn_agent_boot.trn_boot import _ntff_profile_via_ctypes

        h = _ntff_profile_via_ctypes("/opt/axon/libaxon_pjrt.so")
        if h is not None:
            set_axon_ntff_profile_hook(h)


def kernel(**inputs) -> np.ndarray:
    np_inputs = {k: np.asarray(v) for k, v in inputs.items()}
    prep = _prep(**np_inputs)
    K = prep["K"]
    tkey = (K, tuple(sorted(prep["trivial"].items())))
    if tkey not in _CACHE:
        _CACHE[tkey] = _build(K, prep["trivial"])
    nc = _CACHE[tkey]

    in_maps = []
    for c in range(NCORES):
        in_maps.append(dict(
            xsT=prep["xsT"][c], W1h=prep["W1h"], W2h=prep["W2h"],
            idxw1=prep["idxw1"][c], idxw2=prep["idxw2"][c],
            ldt=prep["ldt"][c], epsdegH=prep["epsdegH"][c],
            d1t=prep["d1t"][c], lbt=prep["lbt"][c], WnT=prep["WnT"],
            iota16=prep["iota16"], iotaP16=prep["iotaP16"],
            b1r=prep["b1"][None, :], b2r=prep["b2"][None, :],
            ln1wr=prep["ln1_w"][None, :], ln1br=prep["ln1_b"][None, :],
            ln2wr=prep["ln2_w"][None, :], ln2br=prep["ln2_b"][None, :],
        ))
    import os
    trace = bool(os.environ.get("BASS_KERNEL_TRACE"))
    if trace:
        _ensure_ntff_hook()
    res = run_bass_kernel_spmd(nc, in_maps, core_ids=list(range(NCORES)),
                               trace=trace)
    global LAST_RESULTS
    LAST_RESULTS = res
    if trace and res.exec_time_ns is not None:
        print(f"HW exec time: {res.exec_time_ns} ns", flush=True)

    # host unshard: scatter-add partial logits by per-core graph base,
    # divide by graph node counts, add classifier bias
    out = np.zeros((G, C), np.float64)
    for c in range(NCORES):
        part = res.results[c]["out_part"].astype(np.float64)
        gb = int(prep["g_base"][c])
        hi = min(G, gb + P)
        out[gb:hi] += part[: hi - gb]
    out = out / prep["cnt"][:, None] + prep["cls_b"][None, :]
    return out.astype(np.float32)
